# revision 48
# baseline (speedup 1.0000x reference)
"""Trainium2 Bass kernel: unnormalized single-head attention block.

Computes, for x [4, 4096, 1024] and w_q/w_k/w_v/w_o [1024, 1024] (all fp32):
    q = x @ w_q ; k = x @ w_k ; v = x @ w_v
    scores = q @ k.T            (no softmax)
    out = (scores @ v) @ w_o

Because there is no softmax, the chain is associative and collapses to
    out_b = x_b @ [ w_q @ w_k.T @ (x_b.T @ x_b) @ w_v @ w_o ]
which replaces the two T x T matmuls (34 GFLOP each per batch) with a
Gram matrix G_b = x_b.T @ x_b and a short chain of 1024^3 matmuls:
~90 GFLOP total instead of ~412 GFLOP.

Sharding: 8 NeuronCores = (4 batches) x (2 sequence halves). Each core
computes G over its own 2048-row half; the pair's halves are summed with a
pairwise bf16 AllReduce over groups [[0,1],[2,3],[4,5],[6,7]].

Schedule (PE order), tuned so the tensor engine never waits on the wire:
  1. ~16 dummy matmuls on a zeroed tile warm the HAM clock gate while the
     first x tiles are still in flight (PE would otherwise run its first
     ~3.4us at 1.2 GHz).
  2. G upper triangle only (G is symmetric): per 128-row tile jt, compute
     cols >= 128*jt (56% of the columns). Rows are staged packed into a
     1.125 MB triangle buffer; one AllReduce sums own+peer triangles.
  3. While the collective runs: AT = w_k @ w_q.T and C = w_v @ w_o
     (batch-independent, duplicated on every core -- cheaper than a second
     exchange and exactly fills the collective window).
  4. Post-collective: load the summed triangle, rebuild the 28 lower lhsT
     tiles with PE transposes (row 7 of R needs none, so it is emitted
     first to absorb the collective's exit-barrier latency).
  5. R = G @ C, M = AT.T @ R, out = x_own @ M (psum [t, e] written straight
     to the output layout; stores alternate scalar/sync DMA queues).

Device math is bf16 with fp32 PSUM accumulation (rel err ~5.7e-3 vs fp32
reference). The host ships bf16 tensors directly (x half in both natural
and transposed layout; w_q/w_k/w_v transposed) so no on-device layout
changes or casts are needed.
"""

import contextlib
import ctypes
import os
import sys
import types

import numpy as np

B = 4
T = 4096
D = 1024
H = T // 2          # rows per core
P = 128             # SBUF partitions
NCORES = 8
DT = D // P         # 8 tiles along any 1024 dim
TT = H // P         # 16 own-half t-tiles
FREE = 512          # matmul moving free dim / PSUM bank width (fp32)
KC = D // FREE      # 2 free-dim chunks of 512 along a 1024 dim
GROUPS = [[0, 1], [2, 3], [4, 5], [6, 7]]
NCHUNK = 1     # G-AllReduce chunk count (>1 measured slower: per-collective floors)
CCKIND = os.environ.get("K_CCKIND", "AR")  # AR=AllReduce, AG=AllGather+local add
WARMUP = 16    # dummy matmuls to warm the HAM clock gate during the first DMAs
GSYM = 3       # 3 = triangular G + packed-triangle AllReduce + post-AR transposes

_STATE = {}
LAST_RESULTS = None


def _install_axon_ntff_shim():
    """bass_utils(trace=True) under axon imports antenv.axon_hooks, which the
    agent image lacks. Provide the documented ctypes equivalent so tracing
    works; degrades to hook=None when the .so has no profile symbols."""
    try:
        import antenv.axon_hooks  # noqa: F401
        return
    except ImportError:
        pass

    so_path = "/opt/axon/libaxon_pjrt.so"

    def _make_hook():
        try:
            lib = ctypes.CDLL(so_path)
        except OSError:
            return None
        if not hasattr(lib, "axon_start_nrt_profile"):
            return None
        lib.axon_start_nrt_profile.argtypes = [
            ctypes.POINTER(ctypes.c_int64),
            ctypes.c_size_t,
        ]
        lib.axon_start_nrt_profile.restype = ctypes.c_int64
        lib.axon_stop_nrt_profile.argtypes = [ctypes.c_char_p]
        lib.axon_stop_nrt_profile.restype = ctypes.c_int64

        @contextlib.contextmanager
        def _hook(output_dir, device_ids):
            import jax

            jax.devices()
            if device_ids:
                ids = (ctypes.c_int64 * len(device_ids))(*device_ids)
                rc = lib.axon_start_nrt_profile(ids, len(device_ids))
            else:
                rc = lib.axon_start_nrt_profile(None, 0)
            if rc != 0:
                raise RuntimeError(f"axon_start_nrt_profile rc={rc}")
            try:
                yield
            finally:
                n = lib.axon_stop_nrt_profile(str(output_dir).encode())
                print(f"profile: {n} file(s) written to {output_dir}", file=sys.stderr)

        return _hook

    mod = types.ModuleType("antenv.axon_hooks")
    mod.get_axon_ntff_profile_hook = _make_hook
    mod.set_axon_ntff_profile_hook = lambda h: None
    sys.modules["antenv.axon_hooks"] = mod


def _trace_kernel(tc, xn, xt, wqT, wkT, wvT, wo, out):
    import concourse.mybir as mybir
    from concourse.bass import ts

    nc = tc.nc
    f32 = mybir.dt.float32
    bf16 = mybir.dt.bfloat16

    with contextlib.ExitStack() as top:
        ps_pool = top.enter_context(tc.tile_pool(name="ps", bufs=8, space="PSUM"))
        dram_pool = top.enter_context(tc.tile_pool(name="cdram", bufs=2, space="DRAM"))
        at_pool = top.enter_context(tc.tile_pool(name="at", bufs=DT))
        c_pool = top.enter_context(tc.tile_pool(name="c", bufs=DT))

        # Collective staging in local DRAM (pair groups need Local addr space).
        # The pairwise G AllReduce can be split into chunks so early G rows
        # are in flight while later ones are still computing.
        HB = DT // NCHUNK
        if GSYM == 3:
            # Packed upper-triangle staging: row jt contributes cols >= jt*128.
            TRI_OFF = [0] * DT
            for r in range(1, DT):
                TRI_OFF[r] = TRI_OFF[r - 1] + (DT - (r - 1)) * P
            TRI_W = TRI_OFF[-1] + P  # 4608
            gsrc_tri = dram_pool.tile([P, TRI_W], bf16, name="gsrct", tag="gsrc")
            if CCKIND == "AG":
                gagg_tri = dram_pool.tile(
                    [2, P, TRI_W], bf16, name="gaggt", tag="gsum"
                )
            else:
                gsum_tri = dram_pool.tile([P, TRI_W], bf16, name="gsumt", tag="gsum")
        gsrc = [
            dram_pool.tile([HB, P, D], bf16, name=f"gsrc{h}", tag="gsrc")
            for h in range(NCHUNK)
        ]
        if CCKIND == "AG":
            gagg = [
                dram_pool.tile([2, HB, P, D], bf16, name=f"gagg{h}", tag="gagg")
                for h in range(NCHUNK)
            ]
        else:
            gsum = [
                dram_pool.tile([HB, P, D], bf16, name=f"gsum{h}", tag="gsum")
                for h in range(NCHUNK)
            ]

        if GSYM:
            from concourse import masks

            id_pool = top.enter_context(tc.tile_pool(name="idp", bufs=1))
            ident = id_pool.tile([P, P], bf16, name="ident", tag="id")
            masks.make_identity(nc, ident[:])

        if WARMUP:
            wu_pool = top.enter_context(tc.tile_pool(name="wu", bufs=1))
            wu = wu_pool.tile([P, FREE], bf16, name="wu", tag="wu")
            nc.vector.memset(wu[:], 0.0)
            wps = ps_pool.tile([P, FREE], f32, name="wps", tag="ps")
            for _ in range(WARMUP):
                nc.tensor.matmul(wps[:], wu[:, :P], wu[:], start=True, stop=True)

        with contextlib.ExitStack() as setup:
            xn_pool = setup.enter_context(tc.tile_pool(name="xn", bufs=TT))
            w_pool = setup.enter_context(tc.tile_pool(name="w", bufs=4 * DT))
            gown_pool = setup.enter_context(tc.tile_pool(name="gown", bufs=DT))

            xns = []
            for t in range(TT):
                xv = xn_pool.tile([P, D], bf16, name=f"xn{t}", tag="xn")
                # Alternate queues: G's accumulation needs all 16 tiles, and a
                # single queue streams them slower than the PE consumes them.
                eng = nc.sync if t % 2 == 0 else nc.scalar
                eng.dma_start(out=xv[:], in_=xn[ts(t, P), :])
                xns.append(xv)

            def load_w(w_ap, tag):
                tiles = []
                for i in range(DT):
                    wt = w_pool.tile([P, D], bf16, name=f"{tag}{i}", tag="w")
                    nc.sync.dma_start(out=wt[:], in_=w_ap[ts(i, P), :])
                    tiles.append(wt)
                return tiles

            wk_t = load_w(wkT, "wk")
            wq_t = load_w(wqT, "wq")
            wv_t = load_w(wvT, "wv")
            wo_t = load_w(wo, "wo")

            # --- own-half Gram matrix G[j,k] = sum_t x[t,j] x[t,k] ---
            # G is symmetric: with GSYM, only the upper-triangle blocks are
            # computed with matmuls; the lower tiles are PE-transposes of the
            # upper ones (locally for GSYM 1/2, post-collective for GSYM 3).
            gown = [
                gown_pool.tile([P, D], bf16, name=f"go{j}", tag="gown")
                for j in range(DT)
            ]
            for jt in range(DT):
                if GSYM == 2:
                    # Per-128-tile triangular: compute cols >= jt*128 only.
                    off = jt * P
                    while off < D:
                        w = min(FREE, D - off)
                        psum = ps_pool.tile([P, w], f32, name="psg", tag="ps")
                        for t in range(TT):
                            nc.tensor.matmul(
                                psum[:],
                                xns[t][:, ts(jt, P)],
                                xns[t][:, off : off + w],
                                start=(t == 0),
                                stop=(t == TT - 1),
                            )
                        nc.vector.tensor_copy(gown[jt][:, off : off + w], psum[:])
                        off += w
                elif GSYM == 3:
                    # Triangle only; lower tiles are rebuilt after the AR.
                    off = jt * P
                    while off < D:
                        w = min(FREE, D - off)
                        psum = ps_pool.tile([P, w], f32, name="psg", tag="ps")
                        for t in range(TT):
                            nc.tensor.matmul(
                                psum[:],
                                xns[t][:, ts(jt, P)],
                                xns[t][:, off : off + w],
                                start=(t == 0),
                                stop=(t == TT - 1),
                            )
                        nc.vector.tensor_copy(gown[jt][:, off : off + w], psum[:])
                        off += w
                    nc.scalar.dma_start(
                        out=gsrc_tri[:, TRI_OFF[jt] : TRI_OFF[jt] + (DT - jt) * P],
                        in_=gown[jt][:, jt * P :],
                    )
                    if jt == DT - 1:
                        if CCKIND == "AG":
                            nc.gpsimd.collective_compute(
                                "AllGather",
                                mybir.AluOpType.bypass,
                                replica_groups=GROUPS,
                                ins=[gsrc_tri.opt()],
                                outs=[gagg_tri.opt()],
                            )
                        else:
                            nc.gpsimd.collective_compute(
                                "AllReduce",
                                mybir.AluOpType.add,
                                replica_groups=GROUPS,
                                ins=[gsrc_tri.opt()],
                                outs=[gsum_tri.opt()],
                            )
                    continue
                if GSYM == 2:
                    b0 = 0
                    while b0 < jt:  # lower tiles = transposed earlier rows
                        nb = min(FREE // P, jt - b0)
                        pst = ps_pool.tile([P, nb * P], bf16, name="pst", tag="ps")
                        for i in range(nb):
                            nc.tensor.transpose(
                                pst[:, ts(i, P)],
                                gown[b0 + i][:, ts(jt, P)],
                                ident[:],
                            )
                        nc.vector.tensor_copy(
                            gown[jt][:, b0 * P : (b0 + nb) * P], pst[:]
                        )
                        b0 += nb
                else:
                    lower = GSYM and jt >= DT // 2
                    for kc in ([1] if lower else range(KC)):
                        psum = ps_pool.tile([P, FREE], f32, name="psg", tag="ps")
                        for t in range(TT):
                            nc.tensor.matmul(
                                psum[:],
                                xns[t][:, ts(jt, P)],
                                xns[t][:, ts(kc, FREE)],
                                start=(t == 0),
                                stop=(t == TT - 1),
                            )
                        nc.vector.tensor_copy(gown[jt][:, ts(kc, FREE)], psum[:])
                    if lower:
                        a = jt - DT // 2
                        pst = ps_pool.tile([P, FREE], bf16, name="pst", tag="ps")
                        for b in range(DT // 2):
                            nc.tensor.transpose(
                                pst[:, ts(b, P)],
                                gown[b][:, FREE + a * P : FREE + (a + 1) * P],
                                ident[:],
                            )
                        nc.vector.tensor_copy(gown[jt][:, 0:FREE], pst[:])
                nc.scalar.dma_start(out=gsrc[jt // HB][jt % HB], in_=gown[jt][:])
                if jt % HB == HB - 1:
                    h = jt // HB
                    # Pair exchange of this chunk of G rows.
                    if CCKIND == "AG":
                        nc.gpsimd.collective_compute(
                            "AllGather",
                            mybir.AluOpType.bypass,
                            replica_groups=GROUPS,
                            ins=[gsrc[h].opt()],
                            outs=[gagg[h].opt()],
                        )
                    else:
                        nc.gpsimd.collective_compute(
                            "AllReduce",
                            mybir.AluOpType.add,
                            replica_groups=GROUPS,
                            ins=[gsrc[h].opt()],
                            outs=[gsum[h].opt()],
                        )

            # --- batch-independent products, overlapped with the collective ---
            # AT[j,d] = (w_q @ w_k.T).T = sum_i wk[j,i] wq[d,i]
            ats = [
                at_pool.tile([P, D], bf16, name=f"at{j}", tag="at") for j in range(DT)
            ]
            for jt in range(DT):
                for dc in range(KC):
                    psum = ps_pool.tile([P, FREE], f32, name="psa", tag="ps")
                    for i in range(DT):
                        nc.tensor.matmul(
                            psum[:],
                            wk_t[i][:, ts(jt, P)],
                            wq_t[i][:, ts(dc, FREE)],
                            start=(i == 0),
                            stop=(i == DT - 1),
                        )
                    nc.vector.tensor_copy(ats[jt][:, ts(dc, FREE)], psum[:])

            # C[k,e] = (w_v @ w_o)[k,e] = sum_l wv[k,l] wo[l,e]
            cs = [c_pool.tile([P, D], bf16, name=f"c{k}", tag="c") for k in range(DT)]
            for kt in range(DT):
                for ec in range(KC):
                    psum = ps_pool.tile([P, FREE], f32, name="psc", tag="ps")
                    for l in range(DT):
                        nc.tensor.matmul(
                            psum[:],
                            wv_t[l][:, ts(kt, P)],
                            wo_t[l][:, ts(ec, FREE)],
                            start=(l == 0),
                            stop=(l == DT - 1),
                        )
                    nc.vector.tensor_copy(cs[kt][:, ts(ec, FREE)], psum[:])

        # Late-phase pools, created after the setup pools release their SBUF.
        xt_pool = top.enter_context(tc.tile_pool(name="xt", bufs=DT))
        gf_pool = top.enter_context(tc.tile_pool(name="gf", bufs=DT))
        r_pool = top.enter_context(tc.tile_pool(name="r", bufs=DT))
        m_pool = top.enter_context(tc.tile_pool(name="m", bufs=DT))
        ot_pool = top.enter_context(tc.tile_pool(name="ot", bufs=4))

        # x.T tiles for the final out = x @ M matmul.
        xts = []
        for i in range(DT):
            xv = xt_pool.tile([P, H], bf16, name=f"xt{i}", tag="xt")
            nc.sync.dma_start(out=xv[:], in_=xt[ts(i, P), :])
            xts.append(xv)

        # Full G into SBUF (waits on the AllReduce via tile deps; rides the
        # otherwise-idle SWDGE queue so the wait cannot stall the load queues).
        if GSYM == 3:
            # Load the summed triangle rows; rebuild the 28 lower lhsT tiles
            # as PE transposes of the summed upper tiles.
            tl_pool = top.enter_context(tc.tile_pool(name="tl", bufs=DT - 1))
            if CCKIND == "AG":
                gl_pool = top.enter_context(tc.tile_pool(name="gl", bufs=2 * DT))
            gts = []
            for jt in range(DT):
                w = (DT - jt) * P
                gt = gf_pool.tile([P, w], bf16, name=f"gt{jt}", tag="gf")
                # Spread the collective-gated loads over three queues so they
                # drain in parallel right after the collective completes.
                eng = (nc.gpsimd, nc.sync, nc.scalar)[jt % 3]
                if CCKIND == "AG":
                    g0 = gl_pool.tile([P, w], bf16, name=f"g0{jt}", tag="gl")
                    g1 = gl_pool.tile([P, w], bf16, name=f"g1{jt}", tag="gl")
                    eng.dma_start(
                        out=g0[:], in_=gagg_tri[0, :, TRI_OFF[jt] : TRI_OFF[jt] + w]
                    )
                    eng2 = (nc.sync, nc.scalar, nc.gpsimd)[jt % 3]
                    eng2.dma_start(
                        out=g1[:], in_=gagg_tri[1, :, TRI_OFF[jt] : TRI_OFF[jt] + w]
                    )
                    nc.vector.tensor_tensor(
                        gt[:], g0[:], g1[:], mybir.AluOpType.add
                    )
                else:
                    eng.dma_start(
                        out=gt[:], in_=gsum_tri[:, TRI_OFF[jt] : TRI_OFF[jt] + w]
                    )
                gts.append(gt)
            tlow = {}

            def emit_transposes():
                for jt in range(DT - 1):
                    n = DT - 1 - jt
                    tl = tl_pool.tile([P, n * P], bf16, name=f"tl{jt}", tag="tl")
                    b0 = 0
                    while b0 < n:
                        nb = min(FREE // P, n - b0)
                        pst = ps_pool.tile([P, nb * P], bf16, name="pstl", tag="ps")
                        for i in range(nb):
                            nc.tensor.transpose(
                                pst[:, ts(i, P)],
                                gts[jt][:, (b0 + i + 1) * P : (b0 + i + 2) * P],
                                ident[:],
                            )
                        nc.vector.tensor_copy(tl[:, b0 * P : (b0 + nb) * P], pst[:])
                        b0 += nb
                    tlow[jt] = tl

            def g_lhsT(kt, jt):
                if kt <= jt:
                    return gts[kt][:, (jt - kt) * P : (jt - kt + 1) * P]
                return tlow[jt][:, (kt - jt - 1) * P : (kt - jt) * P]

            # Row DT-1 of R uses only upper/diag tiles, so it can run while
            # the lower-tile transposes' PSUM results are still settling.
            r_order = [DT - 1] + list(range(DT - 1))
        else:
            emit_transposes = None
            r_order = list(range(DT))
            gfs = []
            if CCKIND == "AG":
                gl_pool = top.enter_context(tc.tile_pool(name="gl", bufs=2 * DT))
            for kt in range(DT):
                h, i = kt // HB, kt % HB
                gf = gf_pool.tile([P, D], bf16, name=f"gf{kt}", tag="gf")
                if CCKIND == "AG":
                    ga = gl_pool.tile([P, D], bf16, name=f"ga{kt}", tag="gl")
                    gb = gl_pool.tile([P, D], bf16, name=f"gb{kt}", tag="gl")
                    nc.gpsimd.dma_start(out=ga[:], in_=gagg[h][0, i])
                    nc.gpsimd.dma_start(out=gb[:], in_=gagg[h][1, i])
                    nc.vector.tensor_tensor(
                        gf[:], ga[:], gb[:], mybir.AluOpType.add
                    )
                else:
                    nc.gpsimd.dma_start(out=gf[:], in_=gsum[h][i])
                gfs.append(gf)

            def g_lhsT(kt, jt):
                return gfs[kt][:, ts(jt, P)]

        # R[j,e] = (G @ C)[j,e]; G is symmetric so its tiles serve as lhsT.
        rs = [None] * DT
        for pos, jt in enumerate(r_order):
            rt = r_pool.tile([P, D], bf16, name=f"r{jt}", tag="r")
            for ec in range(KC):
                psum = ps_pool.tile([P, FREE], f32, name="psr", tag="ps")
                for kt in range(DT):
                    nc.tensor.matmul(
                        psum[:],
                        g_lhsT(kt, jt),
                        cs[kt][:, ts(ec, FREE)],
                        start=(kt == 0),
                        stop=(kt == DT - 1),
                    )
                nc.vector.tensor_copy(rt[:, ts(ec, FREE)], psum[:])
            rs[jt] = rt
            if pos == 0 and emit_transposes is not None:
                emit_transposes()

        # M[d,e] = (w_q @ w_k.T @ R)[d,e] = sum_j AT[j,d] R[j,e]
        ms = []
        for dt_ in range(DT):
            mt = m_pool.tile([P, D], bf16, name=f"m{dt_}", tag="m")
            for ec in range(KC):
                psum = ps_pool.tile([P, FREE], f32, name="psm", tag="ps")
                for jt in range(DT):
                    nc.tensor.matmul(
                        psum[:],
                        ats[jt][:, ts(dt_, P)],
                        rs[jt][:, ts(ec, FREE)],
                        start=(jt == 0),
                        stop=(jt == DT - 1),
                    )
                nc.vector.tensor_copy(mt[:, ts(ec, FREE)], psum[:])
            ms.append(mt)

        # out[t,e] = sum_d x[t,d] M[d,e], own-half rows.
        for tt in range(TT):
            for ec in range(KC):
                psum = ps_pool.tile([P, FREE], f32, name="pso", tag="ps")
                for dt_ in range(DT):
                    nc.tensor.matmul(
                        psum[:],
                        xts[dt_][:, ts(tt, P)],
                        ms[dt_][:, ts(ec, FREE)],
                        start=(dt_ == 0),
                        stop=(dt_ == DT - 1),
                    )
                o = ot_pool.tile([P, FREE], f32, name="ot", tag="ot")
                if (tt + ec) % 2 == 0:
                    nc.scalar.copy(o[:], psum[:])
                    nc.scalar.dma_start(out=out[ts(tt, P), ts(ec, FREE)], in_=o[:])
                else:
                    nc.vector.tensor_copy(o[:], psum[:])
                    nc.sync.dma_start(out=out[ts(tt, P), ts(ec, FREE)], in_=o[:])


def _build():
    _install_axon_ntff_shim()
    import concourse.mybir as mybir
    import concourse.tile as tile
    from concourse import bacc

    f32 = mybir.dt.float32
    bf16 = mybir.dt.bfloat16
    nc = bacc.Bacc("TRN2", target_bir_lowering=False, debug=False, num_devices=NCORES)
    xn = nc.dram_tensor("xn", [H, D], bf16, kind="ExternalInput").ap()
    xt = nc.dram_tensor("xt", [D, H], bf16, kind="ExternalInput").ap()
    wqT = nc.dram_tensor("wqT", [D, D], bf16, kind="ExternalInput").ap()
    wkT = nc.dram_tensor("wkT", [D, D], bf16, kind="ExternalInput").ap()
    wvT = nc.dram_tensor("wvT", [D, D], bf16, kind="ExternalInput").ap()
    wo = nc.dram_tensor("wo", [D, D], bf16, kind="ExternalInput").ap()
    out = nc.dram_tensor("out", [H, D], f32, kind="ExternalOutput").ap()

    with tile.TileContext(nc) as tc:
        _trace_kernel(tc, xn, xt, wqT, wkT, wvT, wo, out)
    nc.compile()
    return nc


def kernel(x, w_q, w_k, w_v, w_o):
    global LAST_RESULTS
    import ml_dtypes
    from concourse import bass_utils

    if "nc" not in _STATE:
        _STATE["nc"] = _build()
    nc = _STATE["nc"]

    bf16 = ml_dtypes.bfloat16
    x = np.ascontiguousarray(x, dtype=np.float32)
    wqT = np.asarray(w_q, dtype=np.float32).T.astype(bf16)
    wkT = np.asarray(w_k, dtype=np.float32).T.astype(bf16)
    wvT = np.asarray(w_v, dtype=np.float32).T.astype(bf16)
    wob = np.ascontiguousarray(np.asarray(w_o, dtype=np.float32)).astype(bf16)

    in_maps = []
    for core in range(NCORES):
        b, half = core // 2, core % 2
        xh = x[b, half * H : (half + 1) * H]
        in_maps.append(
            {
                "xn": xh.astype(bf16),
                "xt": xh.T.astype(bf16),
                "wqT": wqT,
                "wkT": wkT,
                "wvT": wvT,
                "wo": wob,
            }
        )

    LAST_RESULTS = bass_utils.run_bass_kernel_spmd(
        nc, in_maps, core_ids=list(range(NCORES))
    )
    out = np.empty((B, T, D), dtype=np.float32)
    for core in range(NCORES):
        b, half = core // 2, core % 2
        out[b, half * H : (half + 1) * H] = LAST_RESULTS.results[core]["out"]
    return out


# revision 50
# speedup vs baseline: 1.1515x; 1.1515x over previous
"""Trainium2 Bass kernel: unnormalized single-head attention block.

Computes, for x [4, 4096, 1024] and w_q/w_k/w_v/w_o [1024, 1024] (all fp32):
    q = x @ w_q ; k = x @ w_k ; v = x @ w_v
    scores = q @ k.T            (no softmax)
    out = (scores @ v) @ w_o

Because there is no softmax, the chain is associative and collapses to
    out_b = x_b @ [ w_q @ w_k.T @ (x_b.T @ x_b) @ w_v @ w_o ]
which replaces the two T x T matmuls (34 GFLOP each per batch) with a
Gram matrix G_b = x_b.T @ x_b and a short chain of 1024^3 matmuls:
~90 GFLOP total instead of ~412 GFLOP.

Sharding: 8 NeuronCores = (4 batches) x (2 sequence halves). Each core
computes G over its own 2048-row half; the pair's halves are summed with a
pairwise bf16 AllReduce over groups [[0,1],[2,3],[4,5],[6,7]].

Schedule (PE order), tuned so the tensor engine never waits on the wire:
  1. ~16 dummy matmuls on a zeroed tile warm the HAM clock gate while the
     first x tiles are still in flight (PE would otherwise run its first
     ~3.4us at 1.2 GHz).
  2. G upper triangle only (G is symmetric): per 128-row tile jt, compute
     cols >= 128*jt (56% of the columns). Rows are staged packed into a
     1.125 MB triangle buffer; one AllReduce sums own+peer triangles.
  3. While the collective runs: AT = w_k @ w_q.T and C = w_v @ w_o
     (batch-independent, duplicated on every core -- cheaper than a second
     exchange and exactly fills the collective window).
  4. Post-collective: load the summed triangle, rebuild the 28 lower lhsT
     tiles with PE transposes (row 7 of R needs none, so it is emitted
     first to absorb the collective's exit-barrier latency).
  5. R = G @ C, M = AT.T @ R, out = x_own @ M (psum [t, e] written straight
     to the output layout; stores alternate scalar/sync DMA queues).

Device math is bf16 with fp32 PSUM accumulation (rel err ~5.7e-3 vs fp32
reference). The host ships bf16 tensors directly (x half in both natural
and transposed layout; w_q/w_k/w_v transposed) so no on-device layout
changes or casts are needed.
"""

import contextlib
import ctypes
import os
import sys
import types

import numpy as np

B = 4
T = 4096
D = 1024
H = T // 2          # rows per core
P = 128             # SBUF partitions
NCORES = 8
DT = D // P         # 8 tiles along any 1024 dim
TT = H // P         # 16 own-half t-tiles
FREE = 512          # matmul moving free dim / PSUM bank width (fp32)
KC = D // FREE      # 2 free-dim chunks of 512 along a 1024 dim
GROUPS = [[0, 1], [2, 3], [4, 5], [6, 7]]
NCHUNK = 1     # G-AllReduce chunk count (>1 measured slower: per-collective floors)
# AllGather + local add has a ~12us wire vs ~35us for AllReduce, but showed a
# nondeterministic NaN (gated loads racing the peer slot's arrival) in 1 of 3
# runs -- AllReduce never failed across 10+ runs, so it stays.
CCKIND = "AR"
WARMUP = 16    # dummy matmuls to warm the HAM clock gate during the first DMAs
GSYM = 3       # 3 = triangular G + packed-triangle AllReduce + post-AR transposes

_STATE = {}
LAST_RESULTS = None


def _install_axon_ntff_shim():
    """bass_utils(trace=True) under axon imports antenv.axon_hooks, which the
    agent image lacks. Provide the documented ctypes equivalent so tracing
    works; degrades to hook=None when the .so has no profile symbols."""
    try:
        import antenv.axon_hooks  # noqa: F401
        return
    except ImportError:
        pass

    so_path = "/opt/axon/libaxon_pjrt.so"

    def _make_hook():
        try:
            lib = ctypes.CDLL(so_path)
        except OSError:
            return None
        if not hasattr(lib, "axon_start_nrt_profile"):
            return None
        lib.axon_start_nrt_profile.argtypes = [
            ctypes.POINTER(ctypes.c_int64),
            ctypes.c_size_t,
        ]
        lib.axon_start_nrt_profile.restype = ctypes.c_int64
        lib.axon_stop_nrt_profile.argtypes = [ctypes.c_char_p]
        lib.axon_stop_nrt_profile.restype = ctypes.c_int64

        @contextlib.contextmanager
        def _hook(output_dir, device_ids):
            import jax

            jax.devices()
            if device_ids:
                ids = (ctypes.c_int64 * len(device_ids))(*device_ids)
                rc = lib.axon_start_nrt_profile(ids, len(device_ids))
            else:
                rc = lib.axon_start_nrt_profile(None, 0)
            if rc != 0:
                raise RuntimeError(f"axon_start_nrt_profile rc={rc}")
            try:
                yield
            finally:
                n = lib.axon_stop_nrt_profile(str(output_dir).encode())
                print(f"profile: {n} file(s) written to {output_dir}", file=sys.stderr)

        return _hook

    mod = types.ModuleType("antenv.axon_hooks")
    mod.get_axon_ntff_profile_hook = _make_hook
    mod.set_axon_ntff_profile_hook = lambda h: None
    sys.modules["antenv.axon_hooks"] = mod


def _trace_kernel(tc, xn, xt, wqT, wkT, wvT, wo, out):
    import concourse.mybir as mybir
    from concourse.bass import ts

    nc = tc.nc
    f32 = mybir.dt.float32
    bf16 = mybir.dt.bfloat16

    with contextlib.ExitStack() as top:
        ps_pool = top.enter_context(tc.tile_pool(name="ps", bufs=8, space="PSUM"))
        dram_pool = top.enter_context(tc.tile_pool(name="cdram", bufs=2, space="DRAM"))
        at_pool = top.enter_context(tc.tile_pool(name="at", bufs=DT))
        c_pool = top.enter_context(tc.tile_pool(name="c", bufs=DT))

        # Collective staging in local DRAM (pair groups need Local addr space).
        # The pairwise G AllReduce can be split into chunks so early G rows
        # are in flight while later ones are still computing.
        HB = DT // NCHUNK
        if GSYM == 3:
            # Packed upper-triangle staging: row jt contributes cols >= jt*128.
            TRI_OFF = [0] * DT
            for r in range(1, DT):
                TRI_OFF[r] = TRI_OFF[r - 1] + (DT - (r - 1)) * P
            TRI_W = TRI_OFF[-1] + P  # 4608
            gsrc_tri = dram_pool.tile([P, TRI_W], bf16, name="gsrct", tag="gsrc")
            if CCKIND == "AG":
                gagg_tri = dram_pool.tile(
                    [2, P, TRI_W], bf16, name="gaggt", tag="gsum"
                )
            else:
                gsum_tri = dram_pool.tile([P, TRI_W], bf16, name="gsumt", tag="gsum")
        gsrc = [
            dram_pool.tile([HB, P, D], bf16, name=f"gsrc{h}", tag="gsrc")
            for h in range(NCHUNK)
        ]
        if CCKIND == "AG":
            gagg = [
                dram_pool.tile([2, HB, P, D], bf16, name=f"gagg{h}", tag="gagg")
                for h in range(NCHUNK)
            ]
        else:
            gsum = [
                dram_pool.tile([HB, P, D], bf16, name=f"gsum{h}", tag="gsum")
                for h in range(NCHUNK)
            ]

        if GSYM:
            from concourse import masks

            id_pool = top.enter_context(tc.tile_pool(name="idp", bufs=1))
            ident = id_pool.tile([P, P], bf16, name="ident", tag="id")
            masks.make_identity(nc, ident[:])

        if WARMUP:
            wu_pool = top.enter_context(tc.tile_pool(name="wu", bufs=1))
            wu = wu_pool.tile([P, FREE], bf16, name="wu", tag="wu")
            nc.vector.memset(wu[:], 0.0)
            wps = ps_pool.tile([P, FREE], f32, name="wps", tag="ps")
            for _ in range(WARMUP):
                nc.tensor.matmul(wps[:], wu[:, :P], wu[:], start=True, stop=True)

        with contextlib.ExitStack() as setup:
            xn_pool = setup.enter_context(tc.tile_pool(name="xn", bufs=TT))
            w_pool = setup.enter_context(tc.tile_pool(name="w", bufs=4 * DT))
            gown_pool = setup.enter_context(tc.tile_pool(name="gown", bufs=DT))

            xns = []
            for t in range(TT):
                xv = xn_pool.tile([P, D], bf16, name=f"xn{t}", tag="xn")
                # Alternate queues: G's accumulation needs all 16 tiles, and a
                # single queue streams them slower than the PE consumes them.
                eng = nc.sync if t % 2 == 0 else nc.scalar
                eng.dma_start(out=xv[:], in_=xn[ts(t, P), :])
                xns.append(xv)

            def load_w(w_ap, tag):
                tiles = []
                for i in range(DT):
                    wt = w_pool.tile([P, D], bf16, name=f"{tag}{i}", tag="w")
                    nc.sync.dma_start(out=wt[:], in_=w_ap[ts(i, P), :])
                    tiles.append(wt)
                return tiles

            wk_t = load_w(wkT, "wk")
            wq_t = load_w(wqT, "wq")
            wv_t = load_w(wvT, "wv")
            wo_t = load_w(wo, "wo")

            # --- own-half Gram matrix G[j,k] = sum_t x[t,j] x[t,k] ---
            # G is symmetric: with GSYM, only the upper-triangle blocks are
            # computed with matmuls; the lower tiles are PE-transposes of the
            # upper ones (locally for GSYM 1/2, post-collective for GSYM 3).
            gown = [
                gown_pool.tile([P, D], bf16, name=f"go{j}", tag="gown")
                for j in range(DT)
            ]
            for jt in range(DT):
                if GSYM == 2:
                    # Per-128-tile triangular: compute cols >= jt*128 only.
                    off = jt * P
                    while off < D:
                        w = min(FREE, D - off)
                        psum = ps_pool.tile([P, w], f32, name="psg", tag="ps")
                        for t in range(TT):
                            nc.tensor.matmul(
                                psum[:],
                                xns[t][:, ts(jt, P)],
                                xns[t][:, off : off + w],
                                start=(t == 0),
                                stop=(t == TT - 1),
                            )
                        nc.vector.tensor_copy(gown[jt][:, off : off + w], psum[:])
                        off += w
                elif GSYM == 3:
                    # Triangle only; lower tiles are rebuilt after the AR.
                    off = jt * P
                    while off < D:
                        w = min(FREE, D - off)
                        psum = ps_pool.tile([P, w], f32, name="psg", tag="ps")
                        for t in range(TT):
                            nc.tensor.matmul(
                                psum[:],
                                xns[t][:, ts(jt, P)],
                                xns[t][:, off : off + w],
                                start=(t == 0),
                                stop=(t == TT - 1),
                            )
                        nc.vector.tensor_copy(gown[jt][:, off : off + w], psum[:])
                        off += w
                    nc.scalar.dma_start(
                        out=gsrc_tri[:, TRI_OFF[jt] : TRI_OFF[jt] + (DT - jt) * P],
                        in_=gown[jt][:, jt * P :],
                    )
                    if jt == DT - 1:
                        if CCKIND == "AG":
                            nc.gpsimd.collective_compute(
                                "AllGather",
                                mybir.AluOpType.bypass,
                                replica_groups=GROUPS,
                                ins=[gsrc_tri.opt()],
                                outs=[gagg_tri.opt()],
                            )
                        else:
                            nc.gpsimd.collective_compute(
                                "AllReduce",
                                mybir.AluOpType.add,
                                replica_groups=GROUPS,
                                ins=[gsrc_tri.opt()],
                                outs=[gsum_tri.opt()],
                            )
                    continue
                if GSYM == 2:
                    b0 = 0
                    while b0 < jt:  # lower tiles = transposed earlier rows
                        nb = min(FREE // P, jt - b0)
                        pst = ps_pool.tile([P, nb * P], bf16, name="pst", tag="ps")
                        for i in range(nb):
                            nc.tensor.transpose(
                                pst[:, ts(i, P)],
                                gown[b0 + i][:, ts(jt, P)],
                                ident[:],
                            )
                        nc.vector.tensor_copy(
                            gown[jt][:, b0 * P : (b0 + nb) * P], pst[:]
                        )
                        b0 += nb
                else:
                    lower = GSYM and jt >= DT // 2
                    for kc in ([1] if lower else range(KC)):
                        psum = ps_pool.tile([P, FREE], f32, name="psg", tag="ps")
                        for t in range(TT):
                            nc.tensor.matmul(
                                psum[:],
                                xns[t][:, ts(jt, P)],
                                xns[t][:, ts(kc, FREE)],
                                start=(t == 0),
                                stop=(t == TT - 1),
                            )
                        nc.vector.tensor_copy(gown[jt][:, ts(kc, FREE)], psum[:])
                    if lower:
                        a = jt - DT // 2
                        pst = ps_pool.tile([P, FREE], bf16, name="pst", tag="ps")
                        for b in range(DT // 2):
                            nc.tensor.transpose(
                                pst[:, ts(b, P)],
                                gown[b][:, FREE + a * P : FREE + (a + 1) * P],
                                ident[:],
                            )
                        nc.vector.tensor_copy(gown[jt][:, 0:FREE], pst[:])
                nc.scalar.dma_start(out=gsrc[jt // HB][jt % HB], in_=gown[jt][:])
                if jt % HB == HB - 1:
                    h = jt // HB
                    # Pair exchange of this chunk of G rows.
                    if CCKIND == "AG":
                        nc.gpsimd.collective_compute(
                            "AllGather",
                            mybir.AluOpType.bypass,
                            replica_groups=GROUPS,
                            ins=[gsrc[h].opt()],
                            outs=[gagg[h].opt()],
                        )
                    else:
                        nc.gpsimd.collective_compute(
                            "AllReduce",
                            mybir.AluOpType.add,
                            replica_groups=GROUPS,
                            ins=[gsrc[h].opt()],
                            outs=[gsum[h].opt()],
                        )

            # --- batch-independent products, overlapped with the collective ---
            # AT[j,d] = (w_q @ w_k.T).T = sum_i wk[j,i] wq[d,i]
            ats = [
                at_pool.tile([P, D], bf16, name=f"at{j}", tag="at") for j in range(DT)
            ]
            for jt in range(DT):
                for dc in range(KC):
                    psum = ps_pool.tile([P, FREE], f32, name="psa", tag="ps")
                    for i in range(DT):
                        nc.tensor.matmul(
                            psum[:],
                            wk_t[i][:, ts(jt, P)],
                            wq_t[i][:, ts(dc, FREE)],
                            start=(i == 0),
                            stop=(i == DT - 1),
                        )
                    nc.vector.tensor_copy(ats[jt][:, ts(dc, FREE)], psum[:])

            # C[k,e] = (w_v @ w_o)[k,e] = sum_l wv[k,l] wo[l,e]
            cs = [c_pool.tile([P, D], bf16, name=f"c{k}", tag="c") for k in range(DT)]
            for kt in range(DT):
                for ec in range(KC):
                    psum = ps_pool.tile([P, FREE], f32, name="psc", tag="ps")
                    for l in range(DT):
                        nc.tensor.matmul(
                            psum[:],
                            wv_t[l][:, ts(kt, P)],
                            wo_t[l][:, ts(ec, FREE)],
                            start=(l == 0),
                            stop=(l == DT - 1),
                        )
                    nc.vector.tensor_copy(cs[kt][:, ts(ec, FREE)], psum[:])

        # Late-phase pools, created after the setup pools release their SBUF.
        xt_pool = top.enter_context(tc.tile_pool(name="xt", bufs=DT))
        gf_pool = top.enter_context(tc.tile_pool(name="gf", bufs=DT))
        r_pool = top.enter_context(tc.tile_pool(name="r", bufs=DT))
        m_pool = top.enter_context(tc.tile_pool(name="m", bufs=DT))
        ot_pool = top.enter_context(tc.tile_pool(name="ot", bufs=4))

        # x.T tiles for the final out = x @ M matmul.
        xts = []
        for i in range(DT):
            xv = xt_pool.tile([P, H], bf16, name=f"xt{i}", tag="xt")
            nc.sync.dma_start(out=xv[:], in_=xt[ts(i, P), :])
            xts.append(xv)

        # Full G into SBUF (waits on the AllReduce via tile deps; rides the
        # otherwise-idle SWDGE queue so the wait cannot stall the load queues).
        if GSYM == 3:
            # Load the summed triangle rows; rebuild the 28 lower lhsT tiles
            # as PE transposes of the summed upper tiles.
            tl_pool = top.enter_context(tc.tile_pool(name="tl", bufs=DT - 1))
            if CCKIND == "AG":
                gl_pool = top.enter_context(tc.tile_pool(name="gl", bufs=2 * DT))
            gts = []
            for jt in range(DT):
                w = (DT - jt) * P
                gt = gf_pool.tile([P, w], bf16, name=f"gt{jt}", tag="gf")
                # Spread the collective-gated loads over three queues so they
                # drain in parallel right after the collective completes.
                eng = (nc.gpsimd, nc.sync, nc.scalar)[jt % 3]
                if CCKIND == "AG":
                    g0 = gl_pool.tile([P, w], bf16, name=f"g0{jt}", tag="gl")
                    g1 = gl_pool.tile([P, w], bf16, name=f"g1{jt}", tag="gl")
                    eng.dma_start(
                        out=g0[:], in_=gagg_tri[0, :, TRI_OFF[jt] : TRI_OFF[jt] + w]
                    )
                    eng2 = (nc.sync, nc.scalar, nc.gpsimd)[jt % 3]
                    eng2.dma_start(
                        out=g1[:], in_=gagg_tri[1, :, TRI_OFF[jt] : TRI_OFF[jt] + w]
                    )
                    nc.vector.tensor_tensor(
                        gt[:], g0[:], g1[:], mybir.AluOpType.add
                    )
                else:
                    eng.dma_start(
                        out=gt[:], in_=gsum_tri[:, TRI_OFF[jt] : TRI_OFF[jt] + w]
                    )
                gts.append(gt)
            tlow = {}

            def emit_transposes():
                for jt in range(DT - 1):
                    n = DT - 1 - jt
                    tl = tl_pool.tile([P, n * P], bf16, name=f"tl{jt}", tag="tl")
                    b0 = 0
                    while b0 < n:
                        nb = min(FREE // P, n - b0)
                        pst = ps_pool.tile([P, nb * P], bf16, name="pstl", tag="ps")
                        for i in range(nb):
                            nc.tensor.transpose(
                                pst[:, ts(i, P)],
                                gts[jt][:, (b0 + i + 1) * P : (b0 + i + 2) * P],
                                ident[:],
                            )
                        nc.vector.tensor_copy(tl[:, b0 * P : (b0 + nb) * P], pst[:])
                        b0 += nb
                    tlow[jt] = tl

            def g_lhsT(kt, jt):
                if kt <= jt:
                    return gts[kt][:, (jt - kt) * P : (jt - kt + 1) * P]
                return tlow[jt][:, (kt - jt - 1) * P : (kt - jt) * P]

            # Row DT-1 of R uses only upper/diag tiles, so it can run while
            # the lower-tile transposes' PSUM results are still settling.
            r_order = [DT - 1] + list(range(DT - 1))
        else:
            emit_transposes = None
            r_order = list(range(DT))
            gfs = []
            if CCKIND == "AG":
                gl_pool = top.enter_context(tc.tile_pool(name="gl", bufs=2 * DT))
            for kt in range(DT):
                h, i = kt // HB, kt % HB
                gf = gf_pool.tile([P, D], bf16, name=f"gf{kt}", tag="gf")
                if CCKIND == "AG":
                    ga = gl_pool.tile([P, D], bf16, name=f"ga{kt}", tag="gl")
                    gb = gl_pool.tile([P, D], bf16, name=f"gb{kt}", tag="gl")
                    nc.gpsimd.dma_start(out=ga[:], in_=gagg[h][0, i])
                    nc.gpsimd.dma_start(out=gb[:], in_=gagg[h][1, i])
                    nc.vector.tensor_tensor(
                        gf[:], ga[:], gb[:], mybir.AluOpType.add
                    )
                else:
                    nc.gpsimd.dma_start(out=gf[:], in_=gsum[h][i])
                gfs.append(gf)

            def g_lhsT(kt, jt):
                return gfs[kt][:, ts(jt, P)]

        # R[j,e] = (G @ C)[j,e]; G is symmetric so its tiles serve as lhsT.
        rs = [None] * DT
        for pos, jt in enumerate(r_order):
            rt = r_pool.tile([P, D], bf16, name=f"r{jt}", tag="r")
            for ec in range(KC):
                psum = ps_pool.tile([P, FREE], f32, name="psr", tag="ps")
                for kt in range(DT):
                    nc.tensor.matmul(
                        psum[:],
                        g_lhsT(kt, jt),
                        cs[kt][:, ts(ec, FREE)],
                        start=(kt == 0),
                        stop=(kt == DT - 1),
                    )
                nc.vector.tensor_copy(rt[:, ts(ec, FREE)], psum[:])
            rs[jt] = rt
            if pos == 0 and emit_transposes is not None:
                emit_transposes()

        # M[d,e] = (w_q @ w_k.T @ R)[d,e] = sum_j AT[j,d] R[j,e]
        ms = []
        for dt_ in range(DT):
            mt = m_pool.tile([P, D], bf16, name=f"m{dt_}", tag="m")
            for ec in range(KC):
                psum = ps_pool.tile([P, FREE], f32, name="psm", tag="ps")
                for jt in range(DT):
                    nc.tensor.matmul(
                        psum[:],
                        ats[jt][:, ts(dt_, P)],
                        rs[jt][:, ts(ec, FREE)],
                        start=(jt == 0),
                        stop=(jt == DT - 1),
                    )
                nc.vector.tensor_copy(mt[:, ts(ec, FREE)], psum[:])
            ms.append(mt)

        # out[t,e] = sum_d x[t,d] M[d,e], own-half rows.
        for tt in range(TT):
            for ec in range(KC):
                psum = ps_pool.tile([P, FREE], f32, name="pso", tag="ps")
                for dt_ in range(DT):
                    nc.tensor.matmul(
                        psum[:],
                        xts[dt_][:, ts(tt, P)],
                        ms[dt_][:, ts(ec, FREE)],
                        start=(dt_ == 0),
                        stop=(dt_ == DT - 1),
                    )
                o = ot_pool.tile([P, FREE], f32, name="ot", tag="ot")
                if (tt + ec) % 2 == 0:
                    nc.scalar.copy(o[:], psum[:])
                    nc.scalar.dma_start(out=out[ts(tt, P), ts(ec, FREE)], in_=o[:])
                else:
                    nc.vector.tensor_copy(o[:], psum[:])
                    nc.sync.dma_start(out=out[ts(tt, P), ts(ec, FREE)], in_=o[:])


def _build():
    _install_axon_ntff_shim()
    import concourse.mybir as mybir
    import concourse.tile as tile
    from concourse import bacc

    f32 = mybir.dt.float32
    bf16 = mybir.dt.bfloat16
    nc = bacc.Bacc("TRN2", target_bir_lowering=False, debug=False, num_devices=NCORES)
    xn = nc.dram_tensor("xn", [H, D], bf16, kind="ExternalInput").ap()
    xt = nc.dram_tensor("xt", [D, H], bf16, kind="ExternalInput").ap()
    wqT = nc.dram_tensor("wqT", [D, D], bf16, kind="ExternalInput").ap()
    wkT = nc.dram_tensor("wkT", [D, D], bf16, kind="ExternalInput").ap()
    wvT = nc.dram_tensor("wvT", [D, D], bf16, kind="ExternalInput").ap()
    wo = nc.dram_tensor("wo", [D, D], bf16, kind="ExternalInput").ap()
    out = nc.dram_tensor("out", [H, D], f32, kind="ExternalOutput").ap()

    with tile.TileContext(nc) as tc:
        _trace_kernel(tc, xn, xt, wqT, wkT, wvT, wo, out)
    nc.compile()
    return nc


def kernel(x, w_q, w_k, w_v, w_o):
    global LAST_RESULTS
    import ml_dtypes
    from concourse import bass_utils

    if "nc" not in _STATE:
        _STATE["nc"] = _build()
    nc = _STATE["nc"]

    bf16 = ml_dtypes.bfloat16
    x = np.ascontiguousarray(x, dtype=np.float32)
    wqT = np.asarray(w_q, dtype=np.float32).T.astype(bf16)
    wkT = np.asarray(w_k, dtype=np.float32).T.astype(bf16)
    wvT = np.asarray(w_v, dtype=np.float32).T.astype(bf16)
    wob = np.ascontiguousarray(np.asarray(w_o, dtype=np.float32)).astype(bf16)

    in_maps = []
    for core in range(NCORES):
        b, half = core // 2, core % 2
        xh = x[b, half * H : (half + 1) * H]
        in_maps.append(
            {
                "xn": xh.astype(bf16),
                "xt": xh.T.astype(bf16),
                "wqT": wqT,
                "wkT": wkT,
                "wvT": wvT,
                "wo": wob,
            }
        )

    LAST_RESULTS = bass_utils.run_bass_kernel_spmd(
        nc, in_maps, core_ids=list(range(NCORES))
    )
    out = np.empty((B, T, D), dtype=np.float32)
    for core in range(NCORES):
        b, half = core // 2, core % 2
        out[b, half * H : (half + 1) * H] = LAST_RESULTS.results[core]["out"]
    return out


# revision 57
# speedup vs baseline: 1.2110x; 1.0517x over previous
"""Trainium2 Bass kernel: unnormalized single-head attention block.

Computes, for x [4, 4096, 1024] and w_q/w_k/w_v/w_o [1024, 1024] (all fp32):
    q = x @ w_q ; k = x @ w_k ; v = x @ w_v
    scores = q @ k.T            (no softmax)
    out = (scores @ v) @ w_o

Because there is no softmax, the chain is associative and collapses to
    out_b = x_b @ [ w_q @ w_k.T @ (x_b.T @ x_b) @ w_v @ w_o ]
which replaces the two T x T matmuls (34 GFLOP each per batch) with a
Gram matrix G_b = x_b.T @ x_b and a short chain of 1024^3 matmuls:
~90 GFLOP total instead of ~412 GFLOP.

Sharding: 8 NeuronCores = (4 batches) x (2 sequence halves). Each core
computes G over its own 2048-row half; the pair's halves are summed with a
pairwise bf16 AllReduce over groups [[0,1],[2,3],[4,5],[6,7]].

Schedule (PE order), tuned so the tensor engine never waits on the wire:
  1. ~16 dummy matmuls on a zeroed tile warm the HAM clock gate while the
     first x tiles are still in flight (PE would otherwise run its first
     ~3.4us at 1.2 GHz).
  2. G upper triangle only (G is symmetric): per 128-row tile jt, compute
     cols >= 128*jt (56% of the columns). Rows are staged packed into a
     1.125 MB triangle buffer; one AllReduce sums own+peer triangles.
  3. While the collective runs: AT = w_k @ w_q.T and C = w_v @ w_o
     (batch-independent, duplicated on every core -- cheaper than a second
     exchange and exactly fills the collective window).
  4. Post-collective: load the summed triangle, rebuild the 28 lower lhsT
     tiles with PE transposes (row 7 of R needs none, so it is emitted
     first to absorb the collective's exit-barrier latency).
  5. R = G @ C, M = AT.T @ R, out = x_own @ M (psum [t, e] written straight
     to the output layout; stores alternate scalar/sync DMA queues).

Device math is bf16 with fp32 PSUM accumulation (rel err ~5.7e-3 vs fp32
reference). The host ships bf16 tensors directly (x half in both natural
and transposed layout; w_q/w_k/w_v transposed) so no on-device layout
changes or casts are needed.
"""

import contextlib
import ctypes
import os
import sys
import types

import numpy as np

B = 4
T = 4096
D = 1024
H = T // 2          # rows per core
P = 128             # SBUF partitions
NCORES = 8
DT = D // P         # 8 tiles along any 1024 dim
TT = H // P         # 16 own-half t-tiles
FREE = 512          # matmul moving free dim / PSUM bank width (fp32)
KC = D // FREE      # 2 free-dim chunks of 512 along a 1024 dim
GROUPS = [[0, 1], [2, 3], [4, 5], [6, 7]]
NCHUNK = 1     # G-AllReduce chunk count (>1 measured slower: per-collective floors)
# AllGather + local add has a ~12us wire vs ~35us for AllReduce, but showed a
# nondeterministic NaN (gated loads racing the peer slot's arrival) in 1 of 3
# runs -- AllReduce never failed across 10+ runs, so it stays.
CCKIND = "AR"
WARMUP = 16    # dummy matmuls to warm the HAM clock gate during the first DMAs
GSYM = 3       # 3 = triangular G + packed-triangle AllReduce + post-AR transposes

_STATE = {}
LAST_RESULTS = None


def _install_axon_ntff_shim():
    """bass_utils(trace=True) under axon imports antenv.axon_hooks, which the
    agent image lacks. Provide the documented ctypes equivalent so tracing
    works; degrades to hook=None when the .so has no profile symbols."""
    try:
        import antenv.axon_hooks  # noqa: F401
        return
    except ImportError:
        pass

    so_path = "/opt/axon/libaxon_pjrt.so"

    def _make_hook():
        try:
            lib = ctypes.CDLL(so_path)
        except OSError:
            return None
        if not hasattr(lib, "axon_start_nrt_profile"):
            return None
        lib.axon_start_nrt_profile.argtypes = [
            ctypes.POINTER(ctypes.c_int64),
            ctypes.c_size_t,
        ]
        lib.axon_start_nrt_profile.restype = ctypes.c_int64
        lib.axon_stop_nrt_profile.argtypes = [ctypes.c_char_p]
        lib.axon_stop_nrt_profile.restype = ctypes.c_int64

        @contextlib.contextmanager
        def _hook(output_dir, device_ids):
            import jax

            jax.devices()
            if device_ids:
                ids = (ctypes.c_int64 * len(device_ids))(*device_ids)
                rc = lib.axon_start_nrt_profile(ids, len(device_ids))
            else:
                rc = lib.axon_start_nrt_profile(None, 0)
            if rc != 0:
                raise RuntimeError(f"axon_start_nrt_profile rc={rc}")
            try:
                yield
            finally:
                n = lib.axon_stop_nrt_profile(str(output_dir).encode())
                print(f"profile: {n} file(s) written to {output_dir}", file=sys.stderr)

        return _hook

    mod = types.ModuleType("antenv.axon_hooks")
    mod.get_axon_ntff_profile_hook = _make_hook
    mod.set_axon_ntff_profile_hook = lambda h: None
    sys.modules["antenv.axon_hooks"] = mod


def _trace_kernel(tc, xn, xt, wqT, wkT, wvT, wo, mask, out):
    import concourse.mybir as mybir
    from concourse.bass import ts

    nc = tc.nc
    f32 = mybir.dt.float32
    bf16 = mybir.dt.bfloat16

    with contextlib.ExitStack() as top:
        ps_pool = top.enter_context(tc.tile_pool(name="ps", bufs=8, space="PSUM"))
        dram_pool = top.enter_context(tc.tile_pool(name="cdram", bufs=2, space="DRAM"))
        at_pool = top.enter_context(tc.tile_pool(name="at", bufs=DT))
        c_pool = top.enter_context(tc.tile_pool(name="c", bufs=DT))

        # Collective staging in local DRAM (pair groups need Local addr space).
        # The pairwise G AllReduce can be split into chunks so early G rows
        # are in flight while later ones are still computing.
        HB = DT // NCHUNK
        if GSYM == 3:
            # Packed upper-triangle staging: row jt contributes cols >= jt*128.
            TRI_OFF = [0] * DT
            for r in range(1, DT):
                TRI_OFF[r] = TRI_OFF[r - 1] + (DT - (r - 1)) * P
            TRI_W = TRI_OFF[-1] + P  # 4608
            gsrc_tri = dram_pool.tile([P, TRI_W], bf16, name="gsrct", tag="gsrc")
            if CCKIND == "AG":
                gagg_tri = dram_pool.tile(
                    [2, P, TRI_W], bf16, name="gaggt", tag="gsum"
                )
            else:
                gsum_tri = dram_pool.tile([P, TRI_W], bf16, name="gsumt", tag="gsum")
        gsrc = [
            dram_pool.tile([HB, P, D], bf16, name=f"gsrc{h}", tag="gsrc")
            for h in range(NCHUNK)
        ]
        if CCKIND == "AG":
            gagg = [
                dram_pool.tile([2, HB, P, D], bf16, name=f"gagg{h}", tag="gagg")
                for h in range(NCHUNK)
            ]
        else:
            gsum = [
                dram_pool.tile([HB, P, D], bf16, name=f"gsum{h}", tag="gsum")
                for h in range(NCHUNK)
            ]

        if GSYM:
            from concourse import masks

            id_pool = top.enter_context(tc.tile_pool(name="idp", bufs=2))
            ident = id_pool.tile([P, P], bf16, name="ident", tag="id")
            masks.make_identity(nc, ident[:])

        # Pair-position mask for the M-half exchange (own slot zeroed), plus
        # the staging/landing buffers for the masked ReduceScatter.
        mb = id_pool.tile([P, 2], f32, name="mb", tag="mb")
        nc.sync.dma_start(out=mb[:], in_=mask)
        mstage = dram_pool.tile([2, DT, P, FREE], bf16, name="mstage", tag="mst")
        mpeer = dram_pool.tile([DT, P, FREE], bf16, name="mpeer", tag="mpr")

        if WARMUP:
            wu_pool = top.enter_context(tc.tile_pool(name="wu", bufs=1))
            wu = wu_pool.tile([P, FREE], bf16, name="wu", tag="wu")
            nc.vector.memset(wu[:], 0.0)
            wps = ps_pool.tile([P, FREE], f32, name="wps", tag="ps")
            for _ in range(WARMUP):
                nc.tensor.matmul(wps[:], wu[:, :P], wu[:], start=True, stop=True)

        with contextlib.ExitStack() as setup:
            xn_pool = setup.enter_context(tc.tile_pool(name="xn", bufs=TT))
            w_pool = setup.enter_context(tc.tile_pool(name="w", bufs=4 * DT))
            gown_pool = setup.enter_context(tc.tile_pool(name="gown", bufs=DT))

            xns = []
            for t in range(TT):
                xv = xn_pool.tile([P, D], bf16, name=f"xn{t}", tag="xn")
                # Alternate queues: G's accumulation needs all 16 tiles, and a
                # single queue streams them slower than the PE consumes them.
                eng = nc.sync if t % 2 == 0 else nc.scalar
                eng.dma_start(out=xv[:], in_=xn[ts(t, P), :])
                xns.append(xv)

            def load_w(w_ap, tag):
                tiles = []
                for i in range(DT):
                    wt = w_pool.tile([P, D], bf16, name=f"{tag}{i}", tag="w")
                    nc.sync.dma_start(out=wt[:], in_=w_ap[ts(i, P), :])
                    tiles.append(wt)
                return tiles

            wk_t = load_w(wkT, "wk")
            wq_t = load_w(wqT, "wq")
            wv_t = load_w(wvT, "wv")
            wo_t = load_w(wo, "wo")

            # --- own-half Gram matrix G[j,k] = sum_t x[t,j] x[t,k] ---
            # G is symmetric: with GSYM, only the upper-triangle blocks are
            # computed with matmuls; the lower tiles are PE-transposes of the
            # upper ones (locally for GSYM 1/2, post-collective for GSYM 3).
            gown = [
                gown_pool.tile([P, D], bf16, name=f"go{j}", tag="gown")
                for j in range(DT)
            ]
            for jt in range(DT):
                if GSYM == 2:
                    # Per-128-tile triangular: compute cols >= jt*128 only.
                    off = jt * P
                    while off < D:
                        w = min(FREE, D - off)
                        psum = ps_pool.tile([P, w], f32, name="psg", tag="ps")
                        for t in range(TT):
                            nc.tensor.matmul(
                                psum[:],
                                xns[t][:, ts(jt, P)],
                                xns[t][:, off : off + w],
                                start=(t == 0),
                                stop=(t == TT - 1),
                            )
                        nc.vector.tensor_copy(gown[jt][:, off : off + w], psum[:])
                        off += w
                elif GSYM == 3:
                    # Triangle only; lower tiles are rebuilt after the AR.
                    off = jt * P
                    while off < D:
                        w = min(FREE, D - off)
                        psum = ps_pool.tile([P, w], f32, name="psg", tag="ps")
                        for t in range(TT):
                            nc.tensor.matmul(
                                psum[:],
                                xns[t][:, ts(jt, P)],
                                xns[t][:, off : off + w],
                                start=(t == 0),
                                stop=(t == TT - 1),
                            )
                        nc.vector.tensor_copy(gown[jt][:, off : off + w], psum[:])
                        off += w
                    nc.scalar.dma_start(
                        out=gsrc_tri[:, TRI_OFF[jt] : TRI_OFF[jt] + (DT - jt) * P],
                        in_=gown[jt][:, jt * P :],
                    )
                    if jt == DT - 1:
                        if CCKIND == "AG":
                            nc.gpsimd.collective_compute(
                                "AllGather",
                                mybir.AluOpType.bypass,
                                replica_groups=GROUPS,
                                ins=[gsrc_tri.opt()],
                                outs=[gagg_tri.opt()],
                            )
                        else:
                            nc.gpsimd.collective_compute(
                                "AllReduce",
                                mybir.AluOpType.add,
                                replica_groups=GROUPS,
                                ins=[gsrc_tri.opt()],
                                outs=[gsum_tri.opt()],
                            )
                    continue
                if GSYM == 2:
                    b0 = 0
                    while b0 < jt:  # lower tiles = transposed earlier rows
                        nb = min(FREE // P, jt - b0)
                        pst = ps_pool.tile([P, nb * P], bf16, name="pst", tag="ps")
                        for i in range(nb):
                            nc.tensor.transpose(
                                pst[:, ts(i, P)],
                                gown[b0 + i][:, ts(jt, P)],
                                ident[:],
                            )
                        nc.vector.tensor_copy(
                            gown[jt][:, b0 * P : (b0 + nb) * P], pst[:]
                        )
                        b0 += nb
                else:
                    lower = GSYM and jt >= DT // 2
                    for kc in ([1] if lower else range(KC)):
                        psum = ps_pool.tile([P, FREE], f32, name="psg", tag="ps")
                        for t in range(TT):
                            nc.tensor.matmul(
                                psum[:],
                                xns[t][:, ts(jt, P)],
                                xns[t][:, ts(kc, FREE)],
                                start=(t == 0),
                                stop=(t == TT - 1),
                            )
                        nc.vector.tensor_copy(gown[jt][:, ts(kc, FREE)], psum[:])
                    if lower:
                        a = jt - DT // 2
                        pst = ps_pool.tile([P, FREE], bf16, name="pst", tag="ps")
                        for b in range(DT // 2):
                            nc.tensor.transpose(
                                pst[:, ts(b, P)],
                                gown[b][:, FREE + a * P : FREE + (a + 1) * P],
                                ident[:],
                            )
                        nc.vector.tensor_copy(gown[jt][:, 0:FREE], pst[:])
                nc.scalar.dma_start(out=gsrc[jt // HB][jt % HB], in_=gown[jt][:])
                if jt % HB == HB - 1:
                    h = jt // HB
                    # Pair exchange of this chunk of G rows.
                    if CCKIND == "AG":
                        nc.gpsimd.collective_compute(
                            "AllGather",
                            mybir.AluOpType.bypass,
                            replica_groups=GROUPS,
                            ins=[gsrc[h].opt()],
                            outs=[gagg[h].opt()],
                        )
                    else:
                        nc.gpsimd.collective_compute(
                            "AllReduce",
                            mybir.AluOpType.add,
                            replica_groups=GROUPS,
                            ins=[gsrc[h].opt()],
                            outs=[gsum[h].opt()],
                        )

            # --- batch-independent products, overlapped with the collective ---
            # AT[j,d] = (w_q @ w_k.T).T = sum_i wk[j,i] wq[d,i]
            ats = [
                at_pool.tile([P, D], bf16, name=f"at{j}", tag="at") for j in range(DT)
            ]
            for jt in range(DT):
                for dc in range(KC):
                    psum = ps_pool.tile([P, FREE], f32, name="psa", tag="ps")
                    for i in range(DT):
                        nc.tensor.matmul(
                            psum[:],
                            wk_t[i][:, ts(jt, P)],
                            wq_t[i][:, ts(dc, FREE)],
                            start=(i == 0),
                            stop=(i == DT - 1),
                        )
                    nc.vector.tensor_copy(ats[jt][:, ts(dc, FREE)], psum[:])

            # C[k,e] = (w_v @ w_o)[k,e] = sum_l wv[k,l] wo[l,e]
            cs = [c_pool.tile([P, D], bf16, name=f"c{k}", tag="c") for k in range(DT)]
            for kt in range(DT):
                for ec in range(KC):
                    psum = ps_pool.tile([P, FREE], f32, name="psc", tag="ps")
                    for l in range(DT):
                        nc.tensor.matmul(
                            psum[:],
                            wv_t[l][:, ts(kt, P)],
                            wo_t[l][:, ts(ec, FREE)],
                            start=(l == 0),
                            stop=(l == DT - 1),
                        )
                    nc.vector.tensor_copy(cs[kt][:, ts(ec, FREE)], psum[:])

        # Late-phase pools, created after the setup pools release their SBUF.
        xt_pool = top.enter_context(tc.tile_pool(name="xt", bufs=DT))
        gf_pool = top.enter_context(tc.tile_pool(name="gf", bufs=DT))
        r_pool = top.enter_context(tc.tile_pool(name="r", bufs=DT))
        m_pool = top.enter_context(tc.tile_pool(name="m", bufs=2 * DT))
        ot_pool = top.enter_context(tc.tile_pool(name="ot", bufs=6))

        # x.T tiles for the final out = x @ M matmul.
        xts = []
        for i in range(DT):
            xv = xt_pool.tile([P, H], bf16, name=f"xt{i}", tag="xt")
            nc.sync.dma_start(out=xv[:], in_=xt[ts(i, P), :])
            xts.append(xv)

        # Full G into SBUF (waits on the AllReduce via tile deps; rides the
        # otherwise-idle SWDGE queue so the wait cannot stall the load queues).
        if GSYM == 3:
            # Load the summed triangle rows; rebuild the 28 lower lhsT tiles
            # as PE transposes of the summed upper tiles.
            tl_pool = top.enter_context(tc.tile_pool(name="tl", bufs=DT - 1))
            if CCKIND == "AG":
                gl_pool = top.enter_context(tc.tile_pool(name="gl", bufs=2 * DT))
            gts = []
            for jt in range(DT):
                w = (DT - jt) * P
                gt = gf_pool.tile([P, w], bf16, name=f"gt{jt}", tag="gf")
                # Spread the collective-gated loads over three queues so they
                # drain in parallel right after the collective completes.
                eng = (nc.gpsimd, nc.sync, nc.scalar)[jt % 3]
                if CCKIND == "AG":
                    g0 = gl_pool.tile([P, w], bf16, name=f"g0{jt}", tag="gl")
                    g1 = gl_pool.tile([P, w], bf16, name=f"g1{jt}", tag="gl")
                    eng.dma_start(
                        out=g0[:], in_=gagg_tri[0, :, TRI_OFF[jt] : TRI_OFF[jt] + w]
                    )
                    eng2 = (nc.sync, nc.scalar, nc.gpsimd)[jt % 3]
                    eng2.dma_start(
                        out=g1[:], in_=gagg_tri[1, :, TRI_OFF[jt] : TRI_OFF[jt] + w]
                    )
                    nc.vector.tensor_tensor(
                        gt[:], g0[:], g1[:], mybir.AluOpType.add
                    )
                else:
                    eng.dma_start(
                        out=gt[:], in_=gsum_tri[:, TRI_OFF[jt] : TRI_OFF[jt] + w]
                    )
                gts.append(gt)
            tlow = {}

            def emit_transposes():
                for jt in range(DT - 1):
                    n = DT - 1 - jt
                    tl = tl_pool.tile([P, n * P], bf16, name=f"tl{jt}", tag="tl")
                    b0 = 0
                    while b0 < n:
                        nb = min(FREE // P, n - b0)
                        pst = ps_pool.tile([P, nb * P], bf16, name="pstl", tag="ps")
                        for i in range(nb):
                            nc.tensor.transpose(
                                pst[:, ts(i, P)],
                                gts[jt][:, (b0 + i + 1) * P : (b0 + i + 2) * P],
                                ident[:],
                            )
                        nc.vector.tensor_copy(tl[:, b0 * P : (b0 + nb) * P], pst[:])
                        b0 += nb
                    tlow[jt] = tl

            def g_lhsT(kt, jt):
                if kt <= jt:
                    return gts[kt][:, (jt - kt) * P : (jt - kt + 1) * P]
                return tlow[jt][:, (kt - jt - 1) * P : (kt - jt) * P]

            # Row DT-1 of R uses only upper/diag tiles, so it can run while
            # the lower-tile transposes' PSUM results are still settling.
            r_order = [DT - 1] + list(range(DT - 1))
        else:
            emit_transposes = None
            r_order = list(range(DT))
            gfs = []
            if CCKIND == "AG":
                gl_pool = top.enter_context(tc.tile_pool(name="gl", bufs=2 * DT))
            for kt in range(DT):
                h, i = kt // HB, kt % HB
                gf = gf_pool.tile([P, D], bf16, name=f"gf{kt}", tag="gf")
                if CCKIND == "AG":
                    ga = gl_pool.tile([P, D], bf16, name=f"ga{kt}", tag="gl")
                    gb = gl_pool.tile([P, D], bf16, name=f"gb{kt}", tag="gl")
                    nc.gpsimd.dma_start(out=ga[:], in_=gagg[h][0, i])
                    nc.gpsimd.dma_start(out=gb[:], in_=gagg[h][1, i])
                    nc.vector.tensor_tensor(
                        gf[:], ga[:], gb[:], mybir.AluOpType.add
                    )
                else:
                    nc.gpsimd.dma_start(out=gf[:], in_=gsum[h][i])
                gfs.append(gf)

            def g_lhsT(kt, jt):
                return gfs[kt][:, ts(jt, P)]

        # R[j,e] = (G @ C)[j,e], own e-half only (host rotated wo so the own
        # half is always cols 0:512). G is symmetric: its tiles serve as lhsT.
        rs = [None] * DT
        for pos, jt in enumerate(r_order):
            rt = r_pool.tile([P, FREE], bf16, name=f"r{jt}", tag="r")
            psum = ps_pool.tile([P, FREE], f32, name="psr", tag="ps")
            for kt in range(DT):
                nc.tensor.matmul(
                    psum[:],
                    g_lhsT(kt, jt),
                    cs[kt][:, 0:FREE],
                    start=(kt == 0),
                    stop=(kt == DT - 1),
                )
            nc.vector.tensor_copy(rt[:], psum[:])
            rs[jt] = rt
            if pos == 0 and emit_transposes is not None:
                emit_transposes()

        # M[d,e] = (w_q @ w_k.T @ R)[d,e], own e-half; the peer computes the
        # other half, exchanged below while out's own half runs on the PE.
        ms = []
        for dt_ in range(DT):
            mt = m_pool.tile([P, FREE], bf16, name=f"m{dt_}", tag="m")
            psum = ps_pool.tile([P, FREE], f32, name="psm", tag="ps")
            for jt in range(DT):
                nc.tensor.matmul(
                    psum[:],
                    ats[jt][:, ts(dt_, P)],
                    rs[jt][:],
                    start=(jt == 0),
                    stop=(jt == DT - 1),
                )
            nc.vector.tensor_copy(mt[:], psum[:])
            ms.append(mt)
            # Masked staging: own slot zeroed, so the pair ReduceScatter
            # delivers exactly the peer's M half on both cores.
            for s in range(2):
                km = ot_pool.tile([P, FREE], bf16, name="km", tag="ot")
                nc.vector.tensor_scalar_mul(km[:], mt[:], mb[:, s : s + 1])
                nc.scalar.dma_start(out=mstage[s, dt_], in_=km[:])
        nc.gpsimd.collective_compute(
            "ReduceScatter",
            mybir.AluOpType.add,
            replica_groups=GROUPS,
            ins=[mstage.opt()],
            outs=[mpeer.opt()],
        )
        mp = []
        for dt_ in range(DT):
            t_ = m_pool.tile([P, FREE], bf16, name=f"mp{dt_}", tag="m")
            nc.gpsimd.dma_start(out=t_[:], in_=mpeer[dt_])
            mp.append(t_)

        # out[t,e] = sum_d x[t,d] M[d,e], own-half rows. The own e-half runs
        # first so the PE is busy while the M exchange is in flight.
        for ec in range(KC):
            src = ms if ec == 0 else mp
            for tt in range(TT):
                psum = ps_pool.tile([P, FREE], f32, name="pso", tag="ps")
                for dt_ in range(DT):
                    nc.tensor.matmul(
                        psum[:],
                        xts[dt_][:, ts(tt, P)],
                        src[dt_][:],
                        start=(dt_ == 0),
                        stop=(dt_ == DT - 1),
                    )
                o = ot_pool.tile([P, FREE], f32, name="ot", tag="ot")
                if (tt + ec) % 2 == 0:
                    nc.scalar.copy(o[:], psum[:])
                    nc.scalar.dma_start(out=out[ts(tt, P), ts(ec, FREE)], in_=o[:])
                else:
                    nc.vector.tensor_copy(o[:], psum[:])
                    nc.sync.dma_start(out=out[ts(tt, P), ts(ec, FREE)], in_=o[:])


def _build():
    _install_axon_ntff_shim()
    import concourse.mybir as mybir
    import concourse.tile as tile
    from concourse import bacc

    f32 = mybir.dt.float32
    bf16 = mybir.dt.bfloat16
    nc = bacc.Bacc("TRN2", target_bir_lowering=False, debug=False, num_devices=NCORES)
    xn = nc.dram_tensor("xn", [H, D], bf16, kind="ExternalInput").ap()
    xt = nc.dram_tensor("xt", [D, H], bf16, kind="ExternalInput").ap()
    wqT = nc.dram_tensor("wqT", [D, D], bf16, kind="ExternalInput").ap()
    wkT = nc.dram_tensor("wkT", [D, D], bf16, kind="ExternalInput").ap()
    wvT = nc.dram_tensor("wvT", [D, D], bf16, kind="ExternalInput").ap()
    wo = nc.dram_tensor("wo", [D, D], bf16, kind="ExternalInput").ap()
    mask = nc.dram_tensor("mask", [P, 2], f32, kind="ExternalInput").ap()
    out = nc.dram_tensor("out", [H, D], f32, kind="ExternalOutput").ap()

    with tile.TileContext(nc) as tc:
        _trace_kernel(tc, xn, xt, wqT, wkT, wvT, wo, mask, out)
    nc.compile()
    return nc


def kernel(x, w_q, w_k, w_v, w_o):
    global LAST_RESULTS
    import ml_dtypes
    from concourse import bass_utils

    if "nc" not in _STATE:
        _STATE["nc"] = _build()
    nc = _STATE["nc"]

    bf16 = ml_dtypes.bfloat16
    x = np.ascontiguousarray(x, dtype=np.float32)
    wqT = np.asarray(w_q, dtype=np.float32).T.astype(bf16)
    wkT = np.asarray(w_k, dtype=np.float32).T.astype(bf16)
    wvT = np.asarray(w_v, dtype=np.float32).T.astype(bf16)
    wob = np.ascontiguousarray(np.asarray(w_o, dtype=np.float32)).astype(bf16)

    # Odd pair members own the upper e-half of the M chain: their wo is
    # column-rotated so "own half" is always cols 0:512 in the SPMD program.
    wob_rot = np.ascontiguousarray(
        np.concatenate([wob[:, D // 2 :], wob[:, : D // 2]], axis=1)
    )
    in_maps = []
    for core in range(NCORES):
        b, half = core // 2, core % 2
        xh = x[b, half * H : (half + 1) * H]
        m = np.zeros((P, 2), dtype=np.float32)
        m[:, 1 - half] = 1.0  # zero own slot; pair position == half
        in_maps.append(
            {
                "xn": xh.astype(bf16),
                "xt": xh.T.astype(bf16),
                "wqT": wqT,
                "wkT": wkT,
                "wvT": wvT,
                "wo": wob if half == 0 else wob_rot,
                "mask": m,
            }
        )

    LAST_RESULTS = bass_utils.run_bass_kernel_spmd(
        nc, in_maps, core_ids=list(range(NCORES))
    )
    out = np.empty((B, T, D), dtype=np.float32)
    for core in range(NCORES):
        b, half = core // 2, core % 2
        res = LAST_RESULTS.results[core]["out"]
        rows = slice(half * H, (half + 1) * H)
        if half == 0:
            out[b, rows] = res
        else:  # un-rotate: rot cols [0:512] are real [512:1024] and vice versa
            out[b, rows, D // 2 :] = res[:, : D // 2]
            out[b, rows, : D // 2] = res[:, D // 2 :]
    return out


# revision 62
# speedup vs baseline: 1.2434x; 1.0268x over previous
"""Trainium2 Bass kernel: unnormalized single-head attention block.

Computes, for x [4, 4096, 1024] and w_q/w_k/w_v/w_o [1024, 1024] (all fp32):
    q = x @ w_q ; k = x @ w_k ; v = x @ w_v
    scores = q @ k.T            (no softmax)
    out = (scores @ v) @ w_o

Because there is no softmax, the chain is associative and collapses to
    out_b = x_b @ [ w_q @ w_k.T @ (x_b.T @ x_b) @ w_v @ w_o ]
which replaces the two T x T matmuls (34 GFLOP each per batch) with a
Gram matrix G_b = x_b.T @ x_b and a short chain of 1024^3 matmuls:
~90 GFLOP total instead of ~412 GFLOP.

Sharding: 8 NeuronCores = (4 batches) x (2 sequence halves). Each core
computes G over its own 2048-row half; the pair's halves are summed with a
pairwise bf16 AllReduce over groups [[0,1],[2,3],[4,5],[6,7]].

Schedule (PE order), tuned so the tensor engine never waits on the wire:
  1. ~16 dummy matmuls on a zeroed tile warm the HAM clock gate while the
     first x tiles are still in flight (PE would otherwise run its first
     ~3.4us at 1.2 GHz).
  2. G upper triangle only (G is symmetric): per 128-row tile jt, compute
     cols >= 128*jt (56% of the columns). Rows are staged packed into a
     1.125 MB triangle buffer; one AllReduce sums own+peer triangles.
  3. While the collective runs: AT = w_k @ w_q.T and C = w_v @ w_o
     (batch-independent, duplicated on every core -- cheaper than a second
     exchange and exactly fills the collective window).
  4. Post-collective: load the summed triangle, rebuild the 28 lower lhsT
     tiles with PE transposes (row 7 of R needs none, so it is emitted
     first to absorb the collective's exit-barrier latency).
  5. R = G @ C and M = AT.T @ R for the OWN e-half only -- the pair splits
     the chain by output column half (host rotates wo's columns per core so
     the own half is always cols 0:512), saving 28 us of duplicated matmuls.
  6. The M halves are exchanged with a masked pair ReduceScatter (own slot
     zeroed, baseline-proven pattern) while out = x_own @ M[:, own] runs on
     the PE; out's peer half follows when the exchange lands. Psum [t, e] is
     written straight to the output layout; stores alternate scalar/sync
     queues; the host un-rotates odd cores' output columns.

Device math is bf16 with fp32 PSUM accumulation (rel err ~5.7e-3 vs fp32
reference). The host ships bf16 tensors directly (x half in both natural
and transposed layout; w_q/w_k/w_v transposed) so no on-device layout
changes or casts are needed.
"""

import contextlib
import ctypes
import os
import sys
import types

import numpy as np

B = 4
T = 4096
D = 1024
H = T // 2          # rows per core
P = 128             # SBUF partitions
NCORES = 8
DT = D // P         # 8 tiles along any 1024 dim
TT = H // P         # 16 own-half t-tiles
FREE = 512          # matmul moving free dim / PSUM bank width (fp32)
KC = D // FREE      # 2 free-dim chunks of 512 along a 1024 dim
GROUPS = [[0, 1], [2, 3], [4, 5], [6, 7]]
NCHUNK = 1     # G-AllReduce chunk count (>1 measured slower: per-collective floors)
# AllGather + local add has a ~12us wire vs ~35us for AllReduce, but showed a
# nondeterministic NaN (gated loads racing the peer slot's arrival) in 1 of 3
# runs -- AllReduce never failed across 10+ runs, so it stays.
CCKIND = "AR"
WARMUP = 16    # dummy matmuls to warm the HAM clock gate during the first DMAs
GSYM = 3       # 3 = triangular G + packed-triangle AllReduce + post-AR transposes

_STATE = {}
LAST_RESULTS = None


def _install_axon_ntff_shim():
    """bass_utils(trace=True) under axon imports antenv.axon_hooks, which the
    agent image lacks. Provide the documented ctypes equivalent so tracing
    works; degrades to hook=None when the .so has no profile symbols."""
    try:
        import antenv.axon_hooks  # noqa: F401
        return
    except ImportError:
        pass

    so_path = "/opt/axon/libaxon_pjrt.so"

    def _make_hook():
        try:
            lib = ctypes.CDLL(so_path)
        except OSError:
            return None
        if not hasattr(lib, "axon_start_nrt_profile"):
            return None
        lib.axon_start_nrt_profile.argtypes = [
            ctypes.POINTER(ctypes.c_int64),
            ctypes.c_size_t,
        ]
        lib.axon_start_nrt_profile.restype = ctypes.c_int64
        lib.axon_stop_nrt_profile.argtypes = [ctypes.c_char_p]
        lib.axon_stop_nrt_profile.restype = ctypes.c_int64

        @contextlib.contextmanager
        def _hook(output_dir, device_ids):
            import jax

            jax.devices()
            if device_ids:
                ids = (ctypes.c_int64 * len(device_ids))(*device_ids)
                rc = lib.axon_start_nrt_profile(ids, len(device_ids))
            else:
                rc = lib.axon_start_nrt_profile(None, 0)
            if rc != 0:
                raise RuntimeError(f"axon_start_nrt_profile rc={rc}")
            try:
                yield
            finally:
                n = lib.axon_stop_nrt_profile(str(output_dir).encode())
                print(f"profile: {n} file(s) written to {output_dir}", file=sys.stderr)

        return _hook

    mod = types.ModuleType("antenv.axon_hooks")
    mod.get_axon_ntff_profile_hook = _make_hook
    mod.set_axon_ntff_profile_hook = lambda h: None
    sys.modules["antenv.axon_hooks"] = mod


def _trace_kernel(tc, xn, xt, wqT, wkT, wvT, wo, mask, out):
    import concourse.mybir as mybir
    from concourse.bass import ts

    nc = tc.nc
    f32 = mybir.dt.float32
    bf16 = mybir.dt.bfloat16

    with contextlib.ExitStack() as top:
        ps_pool = top.enter_context(tc.tile_pool(name="ps", bufs=8, space="PSUM"))
        dram_pool = top.enter_context(tc.tile_pool(name="cdram", bufs=2, space="DRAM"))
        at_pool = top.enter_context(tc.tile_pool(name="at", bufs=DT))
        c_pool = top.enter_context(tc.tile_pool(name="c", bufs=DT))

        # Collective staging in local DRAM (pair groups need Local addr space).
        # The pairwise G AllReduce can be split into chunks so early G rows
        # are in flight while later ones are still computing.
        HB = DT // NCHUNK
        if GSYM == 3:
            # Packed upper-triangle staging: row jt contributes cols >= jt*128.
            TRI_OFF = [0] * DT
            for r in range(1, DT):
                TRI_OFF[r] = TRI_OFF[r - 1] + (DT - (r - 1)) * P
            TRI_W = TRI_OFF[-1] + P  # 4608
            gsrc_tri = dram_pool.tile([P, TRI_W], bf16, name="gsrct", tag="gsrc")
            if CCKIND == "AG":
                gagg_tri = dram_pool.tile(
                    [2, P, TRI_W], bf16, name="gaggt", tag="gsum"
                )
            else:
                gsum_tri = dram_pool.tile([P, TRI_W], bf16, name="gsumt", tag="gsum")
        gsrc = [
            dram_pool.tile([HB, P, D], bf16, name=f"gsrc{h}", tag="gsrc")
            for h in range(NCHUNK)
        ]
        if CCKIND == "AG":
            gagg = [
                dram_pool.tile([2, HB, P, D], bf16, name=f"gagg{h}", tag="gagg")
                for h in range(NCHUNK)
            ]
        else:
            gsum = [
                dram_pool.tile([HB, P, D], bf16, name=f"gsum{h}", tag="gsum")
                for h in range(NCHUNK)
            ]

        if GSYM:
            from concourse import masks

            id_pool = top.enter_context(tc.tile_pool(name="idp", bufs=2))
            ident = id_pool.tile([P, P], bf16, name="ident", tag="id")
            masks.make_identity(nc, ident[:])

        # Pair-position mask for the M-half exchange (own slot zeroed), plus
        # the staging/landing buffers for the masked ReduceScatter.
        mb = id_pool.tile([P, 2], f32, name="mb", tag="mb")
        nc.sync.dma_start(out=mb[:], in_=mask)
        # 75/25 column split of the R/M chain: each core computes rotated
        # cols [0:768); rotated [256:512) is what the peer lacks (with the
        # half-roll rotation both parities send the same rotated slice), and
        # the received chunk lands as rotated cols [768:1024).
        OWN = 3 * D // 4   # 768
        SEND0, SENDW = FREE // 2, FREE // 2  # sent slice [256:512)
        mstage = dram_pool.tile([2, DT, P, SENDW], bf16, name="mstage", tag="mst")
        mpeer = dram_pool.tile([DT, P, SENDW], bf16, name="mpeer", tag="mpr")

        if WARMUP:
            wu_pool = top.enter_context(tc.tile_pool(name="wu", bufs=1))
            wu = wu_pool.tile([P, FREE], bf16, name="wu", tag="wu")
            nc.vector.memset(wu[:], 0.0)
            wps = ps_pool.tile([P, FREE], f32, name="wps", tag="ps")
            for _ in range(WARMUP):
                nc.tensor.matmul(wps[:], wu[:, :P], wu[:], start=True, stop=True)

        with contextlib.ExitStack() as setup:
            xn_pool = setup.enter_context(tc.tile_pool(name="xn", bufs=TT))
            w_pool = setup.enter_context(tc.tile_pool(name="w", bufs=4 * DT))
            gown_pool = setup.enter_context(tc.tile_pool(name="gown", bufs=DT))

            xns = []
            for t in range(TT):
                xv = xn_pool.tile([P, D], bf16, name=f"xn{t}", tag="xn")
                # Alternate queues: G's accumulation needs all 16 tiles, and a
                # single queue streams them slower than the PE consumes them.
                eng = nc.sync if t % 2 == 0 else nc.scalar
                eng.dma_start(out=xv[:], in_=xn[ts(t, P), :])
                xns.append(xv)

            def load_w(w_ap, tag):
                tiles = []
                for i in range(DT):
                    wt = w_pool.tile([P, D], bf16, name=f"{tag}{i}", tag="w")
                    nc.sync.dma_start(out=wt[:], in_=w_ap[ts(i, P), :])
                    tiles.append(wt)
                return tiles

            wk_t = load_w(wkT, "wk")
            wq_t = load_w(wqT, "wq")
            wv_t = load_w(wvT, "wv")
            wo_t = load_w(wo, "wo")

            # --- own-half Gram matrix G[j,k] = sum_t x[t,j] x[t,k] ---
            # G is symmetric: with GSYM, only the upper-triangle blocks are
            # computed with matmuls; the lower tiles are PE-transposes of the
            # upper ones (locally for GSYM 1/2, post-collective for GSYM 3).
            gown = [
                gown_pool.tile([P, D], bf16, name=f"go{j}", tag="gown")
                for j in range(DT)
            ]
            for jt in range(DT):
                if GSYM == 2:
                    # Per-128-tile triangular: compute cols >= jt*128 only.
                    off = jt * P
                    while off < D:
                        w = min(FREE, D - off)
                        psum = ps_pool.tile([P, w], f32, name="psg", tag="ps")
                        for t in range(TT):
                            nc.tensor.matmul(
                                psum[:],
                                xns[t][:, ts(jt, P)],
                                xns[t][:, off : off + w],
                                start=(t == 0),
                                stop=(t == TT - 1),
                            )
                        nc.vector.tensor_copy(gown[jt][:, off : off + w], psum[:])
                        off += w
                elif GSYM == 3:
                    # Triangle only; lower tiles are rebuilt after the AR.
                    off = jt * P
                    while off < D:
                        w = min(FREE, D - off)
                        psum = ps_pool.tile([P, w], f32, name="psg", tag="ps")
                        for t in range(TT):
                            nc.tensor.matmul(
                                psum[:],
                                xns[t][:, ts(jt, P)],
                                xns[t][:, off : off + w],
                                start=(t == 0),
                                stop=(t == TT - 1),
                            )
                        nc.vector.tensor_copy(gown[jt][:, off : off + w], psum[:])
                        off += w
                    nc.scalar.dma_start(
                        out=gsrc_tri[:, TRI_OFF[jt] : TRI_OFF[jt] + (DT - jt) * P],
                        in_=gown[jt][:, jt * P :],
                    )
                    if jt == DT - 1:
                        if CCKIND == "AG":
                            nc.gpsimd.collective_compute(
                                "AllGather",
                                mybir.AluOpType.bypass,
                                replica_groups=GROUPS,
                                ins=[gsrc_tri.opt()],
                                outs=[gagg_tri.opt()],
                            )
                        else:
                            nc.gpsimd.collective_compute(
                                "AllReduce",
                                mybir.AluOpType.add,
                                replica_groups=GROUPS,
                                ins=[gsrc_tri.opt()],
                                outs=[gsum_tri.opt()],
                            )
                    continue
                if GSYM == 2:
                    b0 = 0
                    while b0 < jt:  # lower tiles = transposed earlier rows
                        nb = min(FREE // P, jt - b0)
                        pst = ps_pool.tile([P, nb * P], bf16, name="pst", tag="ps")
                        for i in range(nb):
                            nc.tensor.transpose(
                                pst[:, ts(i, P)],
                                gown[b0 + i][:, ts(jt, P)],
                                ident[:],
                            )
                        nc.vector.tensor_copy(
                            gown[jt][:, b0 * P : (b0 + nb) * P], pst[:]
                        )
                        b0 += nb
                else:
                    lower = GSYM and jt >= DT // 2
                    for kc in ([1] if lower else range(KC)):
                        psum = ps_pool.tile([P, FREE], f32, name="psg", tag="ps")
                        for t in range(TT):
                            nc.tensor.matmul(
                                psum[:],
                                xns[t][:, ts(jt, P)],
                                xns[t][:, ts(kc, FREE)],
                                start=(t == 0),
                                stop=(t == TT - 1),
                            )
                        nc.vector.tensor_copy(gown[jt][:, ts(kc, FREE)], psum[:])
                    if lower:
                        a = jt - DT // 2
                        pst = ps_pool.tile([P, FREE], bf16, name="pst", tag="ps")
                        for b in range(DT // 2):
                            nc.tensor.transpose(
                                pst[:, ts(b, P)],
                                gown[b][:, FREE + a * P : FREE + (a + 1) * P],
                                ident[:],
                            )
                        nc.vector.tensor_copy(gown[jt][:, 0:FREE], pst[:])
                nc.scalar.dma_start(out=gsrc[jt // HB][jt % HB], in_=gown[jt][:])
                if jt % HB == HB - 1:
                    h = jt // HB
                    # Pair exchange of this chunk of G rows.
                    if CCKIND == "AG":
                        nc.gpsimd.collective_compute(
                            "AllGather",
                            mybir.AluOpType.bypass,
                            replica_groups=GROUPS,
                            ins=[gsrc[h].opt()],
                            outs=[gagg[h].opt()],
                        )
                    else:
                        nc.gpsimd.collective_compute(
                            "AllReduce",
                            mybir.AluOpType.add,
                            replica_groups=GROUPS,
                            ins=[gsrc[h].opt()],
                            outs=[gsum[h].opt()],
                        )

            # --- batch-independent products, overlapped with the collective ---
            # AT[j,d] = (w_q @ w_k.T).T = sum_i wk[j,i] wq[d,i]
            ats = [
                at_pool.tile([P, D], bf16, name=f"at{j}", tag="at") for j in range(DT)
            ]
            for jt in range(DT):
                for dc in range(KC):
                    psum = ps_pool.tile([P, FREE], f32, name="psa", tag="ps")
                    for i in range(DT):
                        nc.tensor.matmul(
                            psum[:],
                            wk_t[i][:, ts(jt, P)],
                            wq_t[i][:, ts(dc, FREE)],
                            start=(i == 0),
                            stop=(i == DT - 1),
                        )
                    nc.vector.tensor_copy(ats[jt][:, ts(dc, FREE)], psum[:])

            # C[k,e] = (w_v @ w_o)[k,e] = sum_l wv[k,l] wo[l,e]
            cs = [c_pool.tile([P, D], bf16, name=f"c{k}", tag="c") for k in range(DT)]
            for kt in range(DT):
                for ec in range(KC):
                    psum = ps_pool.tile([P, FREE], f32, name="psc", tag="ps")
                    for l in range(DT):
                        nc.tensor.matmul(
                            psum[:],
                            wv_t[l][:, ts(kt, P)],
                            wo_t[l][:, ts(ec, FREE)],
                            start=(l == 0),
                            stop=(l == DT - 1),
                        )
                    nc.vector.tensor_copy(cs[kt][:, ts(ec, FREE)], psum[:])

        # Late-phase pools, created after the setup pools release their SBUF.
        xt_pool = top.enter_context(tc.tile_pool(name="xt", bufs=DT))
        gf_pool = top.enter_context(tc.tile_pool(name="gf", bufs=DT))
        r_pool = top.enter_context(tc.tile_pool(name="r", bufs=DT))
        m_pool = top.enter_context(tc.tile_pool(name="m", bufs=2 * DT))
        ot_pool = top.enter_context(tc.tile_pool(name="ot", bufs=6))

        # x.T tiles for the final out = x @ M matmul.
        xts = []
        for i in range(DT):
            xv = xt_pool.tile([P, H], bf16, name=f"xt{i}", tag="xt")
            nc.sync.dma_start(out=xv[:], in_=xt[ts(i, P), :])
            xts.append(xv)

        # Full G into SBUF (waits on the AllReduce via tile deps; rides the
        # otherwise-idle SWDGE queue so the wait cannot stall the load queues).
        if GSYM == 3:
            # Load the summed triangle rows; rebuild the 28 lower lhsT tiles
            # as PE transposes of the summed upper tiles.
            tl_pool = top.enter_context(tc.tile_pool(name="tl", bufs=DT - 1))
            if CCKIND == "AG":
                gl_pool = top.enter_context(tc.tile_pool(name="gl", bufs=2 * DT))
            gts = []
            for jt in range(DT):
                w = (DT - jt) * P
                gt = gf_pool.tile([P, w], bf16, name=f"gt{jt}", tag="gf")
                # Spread the collective-gated loads over three queues so they
                # drain in parallel right after the collective completes.
                eng = (nc.gpsimd, nc.sync, nc.scalar)[jt % 3]
                if CCKIND == "AG":
                    g0 = gl_pool.tile([P, w], bf16, name=f"g0{jt}", tag="gl")
                    g1 = gl_pool.tile([P, w], bf16, name=f"g1{jt}", tag="gl")
                    eng.dma_start(
                        out=g0[:], in_=gagg_tri[0, :, TRI_OFF[jt] : TRI_OFF[jt] + w]
                    )
                    eng2 = (nc.sync, nc.scalar, nc.gpsimd)[jt % 3]
                    eng2.dma_start(
                        out=g1[:], in_=gagg_tri[1, :, TRI_OFF[jt] : TRI_OFF[jt] + w]
                    )
                    nc.vector.tensor_tensor(
                        gt[:], g0[:], g1[:], mybir.AluOpType.add
                    )
                else:
                    eng.dma_start(
                        out=gt[:], in_=gsum_tri[:, TRI_OFF[jt] : TRI_OFF[jt] + w]
                    )
                gts.append(gt)
            tlow = {}

            def emit_transposes():
                for jt in range(DT - 1):
                    n = DT - 1 - jt
                    tl = tl_pool.tile([P, n * P], bf16, name=f"tl{jt}", tag="tl")
                    b0 = 0
                    while b0 < n:
                        nb = min(FREE // P, n - b0)
                        pst = ps_pool.tile([P, nb * P], bf16, name="pstl", tag="ps")
                        for i in range(nb):
                            nc.tensor.transpose(
                                pst[:, ts(i, P)],
                                gts[jt][:, (b0 + i + 1) * P : (b0 + i + 2) * P],
                                ident[:],
                            )
                        nc.vector.tensor_copy(tl[:, b0 * P : (b0 + nb) * P], pst[:])
                        b0 += nb
                    tlow[jt] = tl

            def g_lhsT(kt, jt):
                if kt <= jt:
                    return gts[kt][:, (jt - kt) * P : (jt - kt + 1) * P]
                return tlow[jt][:, (kt - jt - 1) * P : (kt - jt) * P]

            # Row DT-1 of R uses only upper/diag tiles, so it can run while
            # the lower-tile transposes' PSUM results are still settling.
            r_order = [DT - 1] + list(range(DT - 1))
        else:
            emit_transposes = None
            r_order = list(range(DT))
            gfs = []
            if CCKIND == "AG":
                gl_pool = top.enter_context(tc.tile_pool(name="gl", bufs=2 * DT))
            for kt in range(DT):
                h, i = kt // HB, kt % HB
                gf = gf_pool.tile([P, D], bf16, name=f"gf{kt}", tag="gf")
                if CCKIND == "AG":
                    ga = gl_pool.tile([P, D], bf16, name=f"ga{kt}", tag="gl")
                    gb = gl_pool.tile([P, D], bf16, name=f"gb{kt}", tag="gl")
                    nc.gpsimd.dma_start(out=ga[:], in_=gagg[h][0, i])
                    nc.gpsimd.dma_start(out=gb[:], in_=gagg[h][1, i])
                    nc.vector.tensor_tensor(
                        gf[:], ga[:], gb[:], mybir.AluOpType.add
                    )
                else:
                    nc.gpsimd.dma_start(out=gf[:], in_=gsum[h][i])
                gfs.append(gf)

            def g_lhsT(kt, jt):
                return gfs[kt][:, ts(jt, P)]

        # R[j,e] = (G @ C)[j,e], own e-half only (host rotated wo so the own
        # half is always cols 0:512). G is symmetric: its tiles serve as lhsT.
        rs = [None] * DT
        for pos, jt in enumerate(r_order):
            rt = r_pool.tile([P, OWN], bf16, name=f"r{jt}", tag="r")
            for off, w in ((0, FREE), (FREE, OWN - FREE)):
                psum = ps_pool.tile([P, w], f32, name="psr", tag="ps")
                for kt in range(DT):
                    nc.tensor.matmul(
                        psum[:],
                        g_lhsT(kt, jt),
                        cs[kt][:, off : off + w],
                        start=(kt == 0),
                        stop=(kt == DT - 1),
                    )
                nc.vector.tensor_copy(rt[:, off : off + w], psum[:])
            rs[jt] = rt
            if pos == 0 and emit_transposes is not None:
                emit_transposes()

        # M[d,e] = (w_q @ w_k.T @ R)[d,e], own e-half; the peer computes the
        # other half, exchanged below while out's own half runs on the PE.
        ms = []
        for dt_ in range(DT):
            mt = m_pool.tile([P, OWN], bf16, name=f"m{dt_}", tag="m")
            for off, w in ((0, FREE), (FREE, OWN - FREE)):
                psum = ps_pool.tile([P, w], f32, name="psm", tag="ps")
                for jt in range(DT):
                    nc.tensor.matmul(
                        psum[:],
                        ats[jt][:, ts(dt_, P)],
                        rs[jt][:, off : off + w],
                        start=(jt == 0),
                        stop=(jt == DT - 1),
                    )
                nc.vector.tensor_copy(mt[:, off : off + w], psum[:])
            ms.append(mt)
            # Masked staging: own slot zeroed, so the pair ReduceScatter
            # delivers exactly the peer's sent slice on both cores.
            for s in range(2):
                km = ot_pool.tile([P, SENDW], bf16, name="km", tag="ot")
                nc.vector.tensor_scalar_mul(
                    km[:], mt[:, SEND0 : SEND0 + SENDW], mb[:, s : s + 1]
                )
                nc.scalar.dma_start(out=mstage[s, dt_], in_=km[:])
        nc.gpsimd.collective_compute(
            "ReduceScatter",
            mybir.AluOpType.add,
            replica_groups=GROUPS,
            ins=[mstage.opt()],
            outs=[mpeer.opt()],
        )
        mp = []
        for dt_ in range(DT):
            t_ = m_pool.tile([P, SENDW], bf16, name=f"mp{dt_}", tag="m")
            nc.gpsimd.dma_start(out=t_[:], in_=mpeer[dt_])
            mp.append(t_)

        # out[t,e] = sum_d x[t,d] M[d,e], own-half rows. The own 768 cols run
        # first so the PE is busy while the M exchange is in flight; the
        # received rotated cols [768:1024) finish last.
        pieces = [(0, FREE, None), (FREE, OWN - FREE, None), (OWN, SENDW, mp)]
        for off, w, src in pieces:
            for tt in range(TT):
                psum = ps_pool.tile([P, w], f32, name="pso", tag="ps")
                for dt_ in range(DT):
                    rhs = src[dt_][:] if src is not None else ms[dt_][:, off : off + w]
                    nc.tensor.matmul(
                        psum[:],
                        xts[dt_][:, ts(tt, P)],
                        rhs,
                        start=(dt_ == 0),
                        stop=(dt_ == DT - 1),
                    )
                o = ot_pool.tile([P, w], f32, name="ot", tag="ot")
                if tt % 2 == 0:
                    nc.scalar.copy(o[:], psum[:])
                    nc.scalar.dma_start(out=out[ts(tt, P), off : off + w], in_=o[:])
                else:
                    nc.vector.tensor_copy(o[:], psum[:])
                    nc.sync.dma_start(out=out[ts(tt, P), off : off + w], in_=o[:])


def _build():
    _install_axon_ntff_shim()
    import concourse.mybir as mybir
    import concourse.tile as tile
    from concourse import bacc

    f32 = mybir.dt.float32
    bf16 = mybir.dt.bfloat16
    nc = bacc.Bacc("TRN2", target_bir_lowering=False, debug=False, num_devices=NCORES)
    xn = nc.dram_tensor("xn", [H, D], bf16, kind="ExternalInput").ap()
    xt = nc.dram_tensor("xt", [D, H], bf16, kind="ExternalInput").ap()
    wqT = nc.dram_tensor("wqT", [D, D], bf16, kind="ExternalInput").ap()
    wkT = nc.dram_tensor("wkT", [D, D], bf16, kind="ExternalInput").ap()
    wvT = nc.dram_tensor("wvT", [D, D], bf16, kind="ExternalInput").ap()
    wo = nc.dram_tensor("wo", [D, D], bf16, kind="ExternalInput").ap()
    mask = nc.dram_tensor("mask", [P, 2], f32, kind="ExternalInput").ap()
    out = nc.dram_tensor("out", [H, D], f32, kind="ExternalOutput").ap()

    with tile.TileContext(nc) as tc:
        _trace_kernel(tc, xn, xt, wqT, wkT, wvT, wo, mask, out)
    nc.compile()
    return nc


def kernel(x, w_q, w_k, w_v, w_o):
    global LAST_RESULTS
    import ml_dtypes
    from concourse import bass_utils

    if "nc" not in _STATE:
        _STATE["nc"] = _build()
    nc = _STATE["nc"]

    bf16 = ml_dtypes.bfloat16
    x = np.ascontiguousarray(x, dtype=np.float32)
    wqT = np.asarray(w_q, dtype=np.float32).T.astype(bf16)
    wkT = np.asarray(w_k, dtype=np.float32).T.astype(bf16)
    wvT = np.asarray(w_v, dtype=np.float32).T.astype(bf16)
    wob = np.ascontiguousarray(np.asarray(w_o, dtype=np.float32)).astype(bf16)

    # Odd pair members own the upper e-half of the M chain: their wo is
    # column-rotated so "own half" is always cols 0:512 in the SPMD program.
    wob_rot = np.ascontiguousarray(
        np.concatenate([wob[:, D // 2 :], wob[:, : D // 2]], axis=1)
    )
    in_maps = []
    for core in range(NCORES):
        b, half = core // 2, core % 2
        xh = x[b, half * H : (half + 1) * H]
        m = np.zeros((P, 2), dtype=np.float32)
        m[:, 1 - half] = 1.0  # zero own slot; pair position == half
        in_maps.append(
            {
                "xn": xh.astype(bf16),
                "xt": xh.T.astype(bf16),
                "wqT": wqT,
                "wkT": wkT,
                "wvT": wvT,
                "wo": wob if half == 0 else wob_rot,
                "mask": m,
            }
        )

    LAST_RESULTS = bass_utils.run_bass_kernel_spmd(
        nc, in_maps, core_ids=list(range(NCORES))
    )
    out = np.empty((B, T, D), dtype=np.float32)
    for core in range(NCORES):
        b, half = core // 2, core % 2
        res = LAST_RESULTS.results[core]["out"]
        rows = slice(half * H, (half + 1) * H)
        if half == 0:
            out[b, rows] = res
        else:  # un-rotate: rot cols [0:512] are real [512:1024] and vice versa
            out[b, rows, D // 2 :] = res[:, : D // 2]
            out[b, rows, : D // 2] = res[:, D // 2 :]
    return out


# revision 63
# speedup vs baseline: 1.2466x; 1.0025x over previous
"""Trainium2 Bass kernel: unnormalized single-head attention block.

Computes, for x [4, 4096, 1024] and w_q/w_k/w_v/w_o [1024, 1024] (all fp32):
    q = x @ w_q ; k = x @ w_k ; v = x @ w_v
    scores = q @ k.T            (no softmax)
    out = (scores @ v) @ w_o

Because there is no softmax, the chain is associative and collapses to
    out_b = x_b @ [ w_q @ w_k.T @ (x_b.T @ x_b) @ w_v @ w_o ]
which replaces the two T x T matmuls (34 GFLOP each per batch) with a
Gram matrix G_b = x_b.T @ x_b and a short chain of 1024^3 matmuls:
~90 GFLOP total instead of ~412 GFLOP.

Sharding: 8 NeuronCores = (4 batches) x (2 sequence halves). Each core
computes G over its own 2048-row half; the pair's halves are summed with a
pairwise bf16 AllReduce over groups [[0,1],[2,3],[4,5],[6,7]].

Schedule (PE order), tuned so the tensor engine never waits on the wire:
  1. ~16 dummy matmuls on a zeroed tile warm the HAM clock gate while the
     first x tiles are still in flight (PE would otherwise run its first
     ~3.4us at 1.2 GHz).
  2. G upper triangle only (G is symmetric): per 128-row tile jt, compute
     cols >= 128*jt (56% of the columns). Rows are staged packed into a
     1.125 MB triangle buffer; one AllReduce sums own+peer triangles.
  3. While the collective runs: AT = w_k @ w_q.T and C = w_v @ w_o
     (batch-independent, duplicated on every core -- cheaper than a second
     exchange and exactly fills the collective window).
  4. Post-collective: load the summed triangle, rebuild the 28 lower lhsT
     tiles with PE transposes (row 7 of R needs none, so it is emitted
     first to absorb the collective's exit-barrier latency).
  5. R = G @ C and M = AT.T @ R for rotated cols [0:768) only -- the pair
     splits the chain 75/25 by output column (the host half-rolls wo's
     columns per core, which makes the slice the peer lacks the SAME
     rotated range [256:512) on both pair members, keeping the SPMD
     program rank-free). 75/25 balances the out-phase's own-column compute
     against the exchange latency so the collective hides completely;
     a 50/50 split left ~23 us of PE idle.
  6. The 256-col M slices are exchanged with a masked pair ReduceScatter
     (own slot zeroed) while out = x_own @ M[:, 0:768] runs on the PE; the
     received cols [768:1024) finish last. Psum [t, e] is written straight
     to the output layout; stores alternate scalar/sync queues; the host
     un-rotates odd cores' output columns.

Device math is bf16 with fp32 PSUM accumulation (rel err ~5.7e-3 vs fp32
reference). The host ships bf16 tensors directly (x half in both natural
and transposed layout; w_q/w_k/w_v transposed) so no on-device layout
changes or casts are needed.
"""

import contextlib
import ctypes
import os
import sys
import types

import numpy as np

B = 4
T = 4096
D = 1024
H = T // 2          # rows per core
P = 128             # SBUF partitions
NCORES = 8
DT = D // P         # 8 tiles along any 1024 dim
TT = H // P         # 16 own-half t-tiles
FREE = 512          # matmul moving free dim / PSUM bank width (fp32)
KC = D // FREE      # 2 free-dim chunks of 512 along a 1024 dim
GROUPS = [[0, 1], [2, 3], [4, 5], [6, 7]]
NCHUNK = 1     # G-AllReduce chunk count (>1 measured slower: per-collective floors)
# AllGather + local add has a ~12us wire vs ~35us for AllReduce, but showed a
# nondeterministic NaN (gated loads racing the peer slot's arrival) in 1 of 3
# runs -- AllReduce never failed across 10+ runs, so it stays.
CCKIND = "AR"
WARMUP = 16    # dummy matmuls to warm the HAM clock gate during the first DMAs
GSYM = 3       # 3 = triangular G + packed-triangle AllReduce + post-AR transposes

_STATE = {}
LAST_RESULTS = None


def _install_axon_ntff_shim():
    """bass_utils(trace=True) under axon imports antenv.axon_hooks, which the
    agent image lacks. Provide the documented ctypes equivalent so tracing
    works; degrades to hook=None when the .so has no profile symbols."""
    try:
        import antenv.axon_hooks  # noqa: F401
        return
    except ImportError:
        pass

    so_path = "/opt/axon/libaxon_pjrt.so"

    def _make_hook():
        try:
            lib = ctypes.CDLL(so_path)
        except OSError:
            return None
        if not hasattr(lib, "axon_start_nrt_profile"):
            return None
        lib.axon_start_nrt_profile.argtypes = [
            ctypes.POINTER(ctypes.c_int64),
            ctypes.c_size_t,
        ]
        lib.axon_start_nrt_profile.restype = ctypes.c_int64
        lib.axon_stop_nrt_profile.argtypes = [ctypes.c_char_p]
        lib.axon_stop_nrt_profile.restype = ctypes.c_int64

        @contextlib.contextmanager
        def _hook(output_dir, device_ids):
            import jax

            jax.devices()
            if device_ids:
                ids = (ctypes.c_int64 * len(device_ids))(*device_ids)
                rc = lib.axon_start_nrt_profile(ids, len(device_ids))
            else:
                rc = lib.axon_start_nrt_profile(None, 0)
            if rc != 0:
                raise RuntimeError(f"axon_start_nrt_profile rc={rc}")
            try:
                yield
            finally:
                n = lib.axon_stop_nrt_profile(str(output_dir).encode())
                print(f"profile: {n} file(s) written to {output_dir}", file=sys.stderr)

        return _hook

    mod = types.ModuleType("antenv.axon_hooks")
    mod.get_axon_ntff_profile_hook = _make_hook
    mod.set_axon_ntff_profile_hook = lambda h: None
    sys.modules["antenv.axon_hooks"] = mod


def _trace_kernel(tc, xn, xt, wqT, wkT, wvT, wo, mask, out):
    import concourse.mybir as mybir
    from concourse.bass import ts

    nc = tc.nc
    f32 = mybir.dt.float32
    bf16 = mybir.dt.bfloat16

    with contextlib.ExitStack() as top:
        ps_pool = top.enter_context(tc.tile_pool(name="ps", bufs=8, space="PSUM"))
        dram_pool = top.enter_context(tc.tile_pool(name="cdram", bufs=2, space="DRAM"))
        at_pool = top.enter_context(tc.tile_pool(name="at", bufs=DT))
        c_pool = top.enter_context(tc.tile_pool(name="c", bufs=DT))

        # Collective staging in local DRAM (pair groups need Local addr space).
        # The pairwise G AllReduce can be split into chunks so early G rows
        # are in flight while later ones are still computing.
        HB = DT // NCHUNK
        if GSYM == 3:
            # Packed upper-triangle staging: row jt contributes cols >= jt*128.
            TRI_OFF = [0] * DT
            for r in range(1, DT):
                TRI_OFF[r] = TRI_OFF[r - 1] + (DT - (r - 1)) * P
            TRI_W = TRI_OFF[-1] + P  # 4608
            gsrc_tri = dram_pool.tile([P, TRI_W], bf16, name="gsrct", tag="gsrc")
            if CCKIND == "AG":
                gagg_tri = dram_pool.tile(
                    [2, P, TRI_W], bf16, name="gaggt", tag="gsum"
                )
            else:
                gsum_tri = dram_pool.tile([P, TRI_W], bf16, name="gsumt", tag="gsum")
        gsrc = [
            dram_pool.tile([HB, P, D], bf16, name=f"gsrc{h}", tag="gsrc")
            for h in range(NCHUNK)
        ]
        if CCKIND == "AG":
            gagg = [
                dram_pool.tile([2, HB, P, D], bf16, name=f"gagg{h}", tag="gagg")
                for h in range(NCHUNK)
            ]
        else:
            gsum = [
                dram_pool.tile([HB, P, D], bf16, name=f"gsum{h}", tag="gsum")
                for h in range(NCHUNK)
            ]

        if GSYM:
            from concourse import masks

            id_pool = top.enter_context(tc.tile_pool(name="idp", bufs=2))
            ident = id_pool.tile([P, P], bf16, name="ident", tag="id")
            masks.make_identity(nc, ident[:])

        # Pair-position mask for the M-half exchange (own slot zeroed), plus
        # the staging/landing buffers for the masked ReduceScatter.
        mb = id_pool.tile([P, 2], f32, name="mb", tag="mb")
        nc.sync.dma_start(out=mb[:], in_=mask)
        # 75/25 column split of the R/M chain: each core computes rotated
        # cols [0:768); rotated [256:512) is what the peer lacks (with the
        # half-roll rotation both parities send the same rotated slice), and
        # the received chunk lands as rotated cols [768:1024).
        OWN = 3 * D // 4   # 768
        SEND0, SENDW = FREE // 2, FREE // 2  # sent slice [256:512)
        mstage = dram_pool.tile([2, DT, P, SENDW], bf16, name="mstage", tag="mst")
        mpeer = dram_pool.tile([DT, P, SENDW], bf16, name="mpeer", tag="mpr")

        if WARMUP:
            wu_pool = top.enter_context(tc.tile_pool(name="wu", bufs=1))
            wu = wu_pool.tile([P, FREE], bf16, name="wu", tag="wu")
            nc.vector.memset(wu[:], 0.0)
            wps = ps_pool.tile([P, FREE], f32, name="wps", tag="ps")
            for _ in range(WARMUP):
                nc.tensor.matmul(wps[:], wu[:, :P], wu[:], start=True, stop=True)

        with contextlib.ExitStack() as setup:
            xn_pool = setup.enter_context(tc.tile_pool(name="xn", bufs=TT))
            w_pool = setup.enter_context(tc.tile_pool(name="w", bufs=4 * DT))
            gown_pool = setup.enter_context(tc.tile_pool(name="gown", bufs=DT))

            xns = []
            for t in range(TT):
                xv = xn_pool.tile([P, D], bf16, name=f"xn{t}", tag="xn")
                # Alternate queues: G's accumulation needs all 16 tiles, and a
                # single queue streams them slower than the PE consumes them.
                eng = nc.sync if t % 2 == 0 else nc.scalar
                eng.dma_start(out=xv[:], in_=xn[ts(t, P), :])
                xns.append(xv)

            def load_w(w_ap, tag):
                tiles = []
                for i in range(DT):
                    wt = w_pool.tile([P, D], bf16, name=f"{tag}{i}", tag="w")
                    nc.sync.dma_start(out=wt[:], in_=w_ap[ts(i, P), :])
                    tiles.append(wt)
                return tiles

            wk_t = load_w(wkT, "wk")
            wq_t = load_w(wqT, "wq")
            wv_t = load_w(wvT, "wv")
            wo_t = load_w(wo, "wo")

            # --- own-half Gram matrix G[j,k] = sum_t x[t,j] x[t,k] ---
            # G is symmetric: with GSYM, only the upper-triangle blocks are
            # computed with matmuls; the lower tiles are PE-transposes of the
            # upper ones (locally for GSYM 1/2, post-collective for GSYM 3).
            gown = [
                gown_pool.tile([P, D], bf16, name=f"go{j}", tag="gown")
                for j in range(DT)
            ]
            for jt in range(DT):
                if GSYM == 2:
                    # Per-128-tile triangular: compute cols >= jt*128 only.
                    off = jt * P
                    while off < D:
                        w = min(FREE, D - off)
                        psum = ps_pool.tile([P, w], f32, name="psg", tag="ps")
                        for t in range(TT):
                            nc.tensor.matmul(
                                psum[:],
                                xns[t][:, ts(jt, P)],
                                xns[t][:, off : off + w],
                                start=(t == 0),
                                stop=(t == TT - 1),
                            )
                        nc.vector.tensor_copy(gown[jt][:, off : off + w], psum[:])
                        off += w
                elif GSYM == 3:
                    # Triangle only; lower tiles are rebuilt after the AR.
                    off = jt * P
                    while off < D:
                        w = min(FREE, D - off)
                        psum = ps_pool.tile([P, w], f32, name="psg", tag="ps")
                        for t in range(TT):
                            nc.tensor.matmul(
                                psum[:],
                                xns[t][:, ts(jt, P)],
                                xns[t][:, off : off + w],
                                start=(t == 0),
                                stop=(t == TT - 1),
                            )
                        nc.vector.tensor_copy(gown[jt][:, off : off + w], psum[:])
                        off += w
                    nc.scalar.dma_start(
                        out=gsrc_tri[:, TRI_OFF[jt] : TRI_OFF[jt] + (DT - jt) * P],
                        in_=gown[jt][:, jt * P :],
                    )
                    if jt == DT - 1:
                        if CCKIND == "AG":
                            nc.gpsimd.collective_compute(
                                "AllGather",
                                mybir.AluOpType.bypass,
                                replica_groups=GROUPS,
                                ins=[gsrc_tri.opt()],
                                outs=[gagg_tri.opt()],
                            )
                        else:
                            nc.gpsimd.collective_compute(
                                "AllReduce",
                                mybir.AluOpType.add,
                                replica_groups=GROUPS,
                                ins=[gsrc_tri.opt()],
                                outs=[gsum_tri.opt()],
                            )
                    continue
                if GSYM == 2:
                    b0 = 0
                    while b0 < jt:  # lower tiles = transposed earlier rows
                        nb = min(FREE // P, jt - b0)
                        pst = ps_pool.tile([P, nb * P], bf16, name="pst", tag="ps")
                        for i in range(nb):
                            nc.tensor.transpose(
                                pst[:, ts(i, P)],
                                gown[b0 + i][:, ts(jt, P)],
                                ident[:],
                            )
                        nc.vector.tensor_copy(
                            gown[jt][:, b0 * P : (b0 + nb) * P], pst[:]
                        )
                        b0 += nb
                else:
                    lower = GSYM and jt >= DT // 2
                    for kc in ([1] if lower else range(KC)):
                        psum = ps_pool.tile([P, FREE], f32, name="psg", tag="ps")
                        for t in range(TT):
                            nc.tensor.matmul(
                                psum[:],
                                xns[t][:, ts(jt, P)],
                                xns[t][:, ts(kc, FREE)],
                                start=(t == 0),
                                stop=(t == TT - 1),
                            )
                        nc.vector.tensor_copy(gown[jt][:, ts(kc, FREE)], psum[:])
                    if lower:
                        a = jt - DT // 2
                        pst = ps_pool.tile([P, FREE], bf16, name="pst", tag="ps")
                        for b in range(DT // 2):
                            nc.tensor.transpose(
                                pst[:, ts(b, P)],
                                gown[b][:, FREE + a * P : FREE + (a + 1) * P],
                                ident[:],
                            )
                        nc.vector.tensor_copy(gown[jt][:, 0:FREE], pst[:])
                nc.scalar.dma_start(out=gsrc[jt // HB][jt % HB], in_=gown[jt][:])
                if jt % HB == HB - 1:
                    h = jt // HB
                    # Pair exchange of this chunk of G rows.
                    if CCKIND == "AG":
                        nc.gpsimd.collective_compute(
                            "AllGather",
                            mybir.AluOpType.bypass,
                            replica_groups=GROUPS,
                            ins=[gsrc[h].opt()],
                            outs=[gagg[h].opt()],
                        )
                    else:
                        nc.gpsimd.collective_compute(
                            "AllReduce",
                            mybir.AluOpType.add,
                            replica_groups=GROUPS,
                            ins=[gsrc[h].opt()],
                            outs=[gsum[h].opt()],
                        )

            # --- batch-independent products, overlapped with the collective ---
            # AT[j,d] = (w_q @ w_k.T).T = sum_i wk[j,i] wq[d,i]
            ats = [
                at_pool.tile([P, D], bf16, name=f"at{j}", tag="at") for j in range(DT)
            ]
            for jt in range(DT):
                for dc in range(KC):
                    psum = ps_pool.tile([P, FREE], f32, name="psa", tag="ps")
                    for i in range(DT):
                        nc.tensor.matmul(
                            psum[:],
                            wk_t[i][:, ts(jt, P)],
                            wq_t[i][:, ts(dc, FREE)],
                            start=(i == 0),
                            stop=(i == DT - 1),
                        )
                    nc.vector.tensor_copy(ats[jt][:, ts(dc, FREE)], psum[:])

            # C[k,e] = (w_v @ w_o)[k,e] = sum_l wv[k,l] wo[l,e]
            cs = [c_pool.tile([P, D], bf16, name=f"c{k}", tag="c") for k in range(DT)]
            for kt in range(DT):
                for ec in range(KC):
                    psum = ps_pool.tile([P, FREE], f32, name="psc", tag="ps")
                    for l in range(DT):
                        nc.tensor.matmul(
                            psum[:],
                            wv_t[l][:, ts(kt, P)],
                            wo_t[l][:, ts(ec, FREE)],
                            start=(l == 0),
                            stop=(l == DT - 1),
                        )
                    nc.vector.tensor_copy(cs[kt][:, ts(ec, FREE)], psum[:])

        # Late-phase pools, created after the setup pools release their SBUF.
        xt_pool = top.enter_context(tc.tile_pool(name="xt", bufs=DT))
        gf_pool = top.enter_context(tc.tile_pool(name="gf", bufs=DT))
        r_pool = top.enter_context(tc.tile_pool(name="r", bufs=DT))
        m_pool = top.enter_context(tc.tile_pool(name="m", bufs=2 * DT))
        ot_pool = top.enter_context(tc.tile_pool(name="ot", bufs=6))

        # x.T tiles for the final out = x @ M matmul.
        xts = []
        for i in range(DT):
            xv = xt_pool.tile([P, H], bf16, name=f"xt{i}", tag="xt")
            nc.sync.dma_start(out=xv[:], in_=xt[ts(i, P), :])
            xts.append(xv)

        # Full G into SBUF (waits on the AllReduce via tile deps; rides the
        # otherwise-idle SWDGE queue so the wait cannot stall the load queues).
        if GSYM == 3:
            # Load the summed triangle rows; rebuild the 28 lower lhsT tiles
            # as PE transposes of the summed upper tiles.
            tl_pool = top.enter_context(tc.tile_pool(name="tl", bufs=DT - 1))
            if CCKIND == "AG":
                gl_pool = top.enter_context(tc.tile_pool(name="gl", bufs=2 * DT))
            gts = []
            for jt in range(DT):
                w = (DT - jt) * P
                gt = gf_pool.tile([P, w], bf16, name=f"gt{jt}", tag="gf")
                # Spread the collective-gated loads over three queues so they
                # drain in parallel right after the collective completes.
                eng = (nc.gpsimd, nc.sync, nc.scalar)[jt % 3]
                if CCKIND == "AG":
                    g0 = gl_pool.tile([P, w], bf16, name=f"g0{jt}", tag="gl")
                    g1 = gl_pool.tile([P, w], bf16, name=f"g1{jt}", tag="gl")
                    eng.dma_start(
                        out=g0[:], in_=gagg_tri[0, :, TRI_OFF[jt] : TRI_OFF[jt] + w]
                    )
                    eng2 = (nc.sync, nc.scalar, nc.gpsimd)[jt % 3]
                    eng2.dma_start(
                        out=g1[:], in_=gagg_tri[1, :, TRI_OFF[jt] : TRI_OFF[jt] + w]
                    )
                    nc.vector.tensor_tensor(
                        gt[:], g0[:], g1[:], mybir.AluOpType.add
                    )
                else:
                    eng.dma_start(
                        out=gt[:], in_=gsum_tri[:, TRI_OFF[jt] : TRI_OFF[jt] + w]
                    )
                gts.append(gt)
            tlow = {}

            def emit_transposes():
                for jt in range(DT - 1):
                    n = DT - 1 - jt
                    tl = tl_pool.tile([P, n * P], bf16, name=f"tl{jt}", tag="tl")
                    b0 = 0
                    while b0 < n:
                        nb = min(FREE // P, n - b0)
                        pst = ps_pool.tile([P, nb * P], bf16, name="pstl", tag="ps")
                        for i in range(nb):
                            nc.tensor.transpose(
                                pst[:, ts(i, P)],
                                gts[jt][:, (b0 + i + 1) * P : (b0 + i + 2) * P],
                                ident[:],
                            )
                        nc.vector.tensor_copy(tl[:, b0 * P : (b0 + nb) * P], pst[:])
                        b0 += nb
                    tlow[jt] = tl

            def g_lhsT(kt, jt):
                if kt <= jt:
                    return gts[kt][:, (jt - kt) * P : (jt - kt + 1) * P]
                return tlow[jt][:, (kt - jt - 1) * P : (kt - jt) * P]

            # Row DT-1 of R uses only upper/diag tiles, so it can run while
            # the lower-tile transposes' PSUM results are still settling.
            r_order = [DT - 1] + list(range(DT - 1))
        else:
            emit_transposes = None
            r_order = list(range(DT))
            gfs = []
            if CCKIND == "AG":
                gl_pool = top.enter_context(tc.tile_pool(name="gl", bufs=2 * DT))
            for kt in range(DT):
                h, i = kt // HB, kt % HB
                gf = gf_pool.tile([P, D], bf16, name=f"gf{kt}", tag="gf")
                if CCKIND == "AG":
                    ga = gl_pool.tile([P, D], bf16, name=f"ga{kt}", tag="gl")
                    gb = gl_pool.tile([P, D], bf16, name=f"gb{kt}", tag="gl")
                    nc.gpsimd.dma_start(out=ga[:], in_=gagg[h][0, i])
                    nc.gpsimd.dma_start(out=gb[:], in_=gagg[h][1, i])
                    nc.vector.tensor_tensor(
                        gf[:], ga[:], gb[:], mybir.AluOpType.add
                    )
                else:
                    nc.gpsimd.dma_start(out=gf[:], in_=gsum[h][i])
                gfs.append(gf)

            def g_lhsT(kt, jt):
                return gfs[kt][:, ts(jt, P)]

        # R[j,e] = (G @ C)[j,e], own e-half only (host rotated wo so the own
        # half is always cols 0:512). G is symmetric: its tiles serve as lhsT.
        rs = [None] * DT
        for pos, jt in enumerate(r_order):
            rt = r_pool.tile([P, OWN], bf16, name=f"r{jt}", tag="r")
            for off, w in ((0, FREE), (FREE, OWN - FREE)):
                psum = ps_pool.tile([P, w], f32, name="psr", tag="ps")
                for kt in range(DT):
                    nc.tensor.matmul(
                        psum[:],
                        g_lhsT(kt, jt),
                        cs[kt][:, off : off + w],
                        start=(kt == 0),
                        stop=(kt == DT - 1),
                    )
                nc.vector.tensor_copy(rt[:, off : off + w], psum[:])
            rs[jt] = rt
            if pos == 0 and emit_transposes is not None:
                emit_transposes()

        # M[d,e] = (w_q @ w_k.T @ R)[d,e], own e-half; the peer computes the
        # other half, exchanged below while out's own half runs on the PE.
        ms = []
        for dt_ in range(DT):
            mt = m_pool.tile([P, OWN], bf16, name=f"m{dt_}", tag="m")
            for off, w in ((0, FREE), (FREE, OWN - FREE)):
                psum = ps_pool.tile([P, w], f32, name="psm", tag="ps")
                for jt in range(DT):
                    nc.tensor.matmul(
                        psum[:],
                        ats[jt][:, ts(dt_, P)],
                        rs[jt][:, off : off + w],
                        start=(jt == 0),
                        stop=(jt == DT - 1),
                    )
                nc.vector.tensor_copy(mt[:, off : off + w], psum[:])
            ms.append(mt)
            # Masked staging: own slot zeroed, so the pair ReduceScatter
            # delivers exactly the peer's sent slice on both cores.
            for s in range(2):
                km = ot_pool.tile([P, SENDW], bf16, name="km", tag="ot")
                nc.vector.tensor_scalar_mul(
                    km[:], mt[:, SEND0 : SEND0 + SENDW], mb[:, s : s + 1]
                )
                nc.scalar.dma_start(out=mstage[s, dt_], in_=km[:])
        nc.gpsimd.collective_compute(
            "ReduceScatter",
            mybir.AluOpType.add,
            replica_groups=GROUPS,
            ins=[mstage.opt()],
            outs=[mpeer.opt()],
        )
        mp = []
        for dt_ in range(DT):
            t_ = m_pool.tile([P, SENDW], bf16, name=f"mp{dt_}", tag="m")
            nc.gpsimd.dma_start(out=t_[:], in_=mpeer[dt_])
            mp.append(t_)

        # out[t,e] = sum_d x[t,d] M[d,e], own-half rows. The own 768 cols run
        # first so the PE is busy while the M exchange is in flight; the
        # received rotated cols [768:1024) finish last.
        pieces = [(0, FREE, None), (FREE, OWN - FREE, None), (OWN, SENDW, mp)]
        for off, w, src in pieces:
            for tt in range(TT):
                psum = ps_pool.tile([P, w], f32, name="pso", tag="ps")
                for dt_ in range(DT):
                    rhs = src[dt_][:] if src is not None else ms[dt_][:, off : off + w]
                    nc.tensor.matmul(
                        psum[:],
                        xts[dt_][:, ts(tt, P)],
                        rhs,
                        start=(dt_ == 0),
                        stop=(dt_ == DT - 1),
                    )
                o = ot_pool.tile([P, w], f32, name="ot", tag="ot")
                if tt % 2 == 0:
                    nc.scalar.copy(o[:], psum[:])
                    nc.scalar.dma_start(out=out[ts(tt, P), off : off + w], in_=o[:])
                else:
                    nc.vector.tensor_copy(o[:], psum[:])
                    nc.sync.dma_start(out=out[ts(tt, P), off : off + w], in_=o[:])


def _build():
    _install_axon_ntff_shim()
    import concourse.mybir as mybir
    import concourse.tile as tile
    from concourse import bacc

    f32 = mybir.dt.float32
    bf16 = mybir.dt.bfloat16
    nc = bacc.Bacc("TRN2", target_bir_lowering=False, debug=False, num_devices=NCORES)
    xn = nc.dram_tensor("xn", [H, D], bf16, kind="ExternalInput").ap()
    xt = nc.dram_tensor("xt", [D, H], bf16, kind="ExternalInput").ap()
    wqT = nc.dram_tensor("wqT", [D, D], bf16, kind="ExternalInput").ap()
    wkT = nc.dram_tensor("wkT", [D, D], bf16, kind="ExternalInput").ap()
    wvT = nc.dram_tensor("wvT", [D, D], bf16, kind="ExternalInput").ap()
    wo = nc.dram_tensor("wo", [D, D], bf16, kind="ExternalInput").ap()
    mask = nc.dram_tensor("mask", [P, 2], f32, kind="ExternalInput").ap()
    out = nc.dram_tensor("out", [H, D], f32, kind="ExternalOutput").ap()

    with tile.TileContext(nc) as tc:
        _trace_kernel(tc, xn, xt, wqT, wkT, wvT, wo, mask, out)
    nc.compile()
    return nc


def kernel(x, w_q, w_k, w_v, w_o):
    global LAST_RESULTS
    import ml_dtypes
    from concourse import bass_utils

    if "nc" not in _STATE:
        _STATE["nc"] = _build()
    nc = _STATE["nc"]

    bf16 = ml_dtypes.bfloat16
    x = np.ascontiguousarray(x, dtype=np.float32)
    wqT = np.asarray(w_q, dtype=np.float32).T.astype(bf16)
    wkT = np.asarray(w_k, dtype=np.float32).T.astype(bf16)
    wvT = np.asarray(w_v, dtype=np.float32).T.astype(bf16)
    wob = np.ascontiguousarray(np.asarray(w_o, dtype=np.float32)).astype(bf16)

    # Odd pair members own the upper e-half of the M chain: their wo is
    # column-rotated so "own half" is always cols 0:512 in the SPMD program.
    wob_rot = np.ascontiguousarray(
        np.concatenate([wob[:, D // 2 :], wob[:, : D // 2]], axis=1)
    )
    in_maps = []
    for core in range(NCORES):
        b, half = core // 2, core % 2
        xh = x[b, half * H : (half + 1) * H]
        m = np.zeros((P, 2), dtype=np.float32)
        m[:, 1 - half] = 1.0  # zero own slot; pair position == half
        in_maps.append(
            {
                "xn": xh.astype(bf16),
                "xt": xh.T.astype(bf16),
                "wqT": wqT,
                "wkT": wkT,
                "wvT": wvT,
                "wo": wob if half == 0 else wob_rot,
                "mask": m,
            }
        )

    LAST_RESULTS = bass_utils.run_bass_kernel_spmd(
        nc, in_maps, core_ids=list(range(NCORES))
    )
    out = np.empty((B, T, D), dtype=np.float32)
    for core in range(NCORES):
        b, half = core // 2, core % 2
        res = LAST_RESULTS.results[core]["out"]
        rows = slice(half * H, (half + 1) * H)
        if half == 0:
            out[b, rows] = res
        else:  # un-rotate: rot cols [0:512] are real [512:1024] and vice versa
            out[b, rows, D // 2 :] = res[:, : D // 2]
            out[b, rows, : D // 2] = res[:, D // 2 :]
    return out
